# revision 33
# baseline (speedup 1.0000x reference)
"""Trainium2 Bass kernel for nn_ChunkProcessor (segment-mean -> 2-layer
transformer encoder over chunks -> gather-expand -> final LN).

Sharding: data-parallel over batch B=8 across the 8 NeuronCores; each core
processes one batch item end to end (no cross-core communication).

v2 design notes (perf):
  - tokens / weights pre-cast to bf16 on HOST; output written bf16 and
    upcast on host: halves all big HBM traffic.
  - 1/counts computed on host -> no count matmuls, no phase-1 reciprocals,
    PSUM banks freed.
  - tokens + output move in 1 MiB group DMAs (8 tiles each) for full DMA BW.
  - attention softmax normalization: denominators from the v ones-column,
    reciprocal_approx_fast on a [2,C] pair tile, broadcast to 128 partitions
    with ONE fp32r matmul per head pair (1 cyc/row), numerators bounced to
    SBUF on ACT, one DVE mult per head. Kills the [1,C] exact reciprocals
    (1.4us each) and fp32 1x64 broadcast matmuls (1us each) of v1.
  - attention software-pipelined: scores for head h+1 are emitted on PE
    before AV of head h so the PE never waits on ACT's exp -- keeps the PE
    HAM clock gate at 2.4 GHz (idle PE re-throttles to 1.2 GHz).
"""

import numpy as np
import ml_dtypes

B, S, D = 8, 8192, 512
C, H, L, DFF = 512, 8, 2, 2048
HD = D // H          # 64
NT = S // 128        # 64 token tiles
CT = C // 128        # 4 chunk tiles
DT = D // 128        # 4 feature tiles
FT = DFF // 128      # 16
GT = 4               # token tiles per DMA group
NG = NT // GT        # token-tile DMA groups
SW = 64.0            # fp8 weight pre-scale (host); folded back as 1/SW
EPS = 1e-5

_CACHE = {}


def _build(flags, ranges):
    """Build the Bass program.

    flags  = (qkv_b, out_b, ff1_b, ff2_b, ln1_aff, ln2_aff, fln_aff) bools.
    ranges = tuple of (lo_m, hi_m) per token tile t: the contiguous range of
             chunk tiles any batch item's tile-t segment ids fall into.
    """
    import concourse.bass as bass
    import concourse.tile as tile
    from concourse import bacc, mybir
    from concourse.masks import make_identity

    (has_qkv_b, has_out_b, has_ff1_b, has_ff2_b,
     has_ln1, has_ln2, has_fln) = flags

    # first/last contributing token tile per chunk tile (for PSUM start/stop)
    first_t = [min(t for t in range(NT) if ranges[t][0] <= m <= ranges[t][1])
               for m in range(CT)]
    last_t = [max(t for t in range(NT) if ranges[t][0] <= m <= ranges[t][1])
              for m in range(CT)]

    f32 = mybir.dt.float32
    f32r = mybir.dt.float32r
    bf16 = mybir.dt.bfloat16
    f16 = mybir.dt.float16
    fp8 = mybir.dt.float8e4
    AL = mybir.AluOpType
    AF = mybir.ActivationFunctionType
    DR = mybir.MatmulPerfMode.DoubleRow

    nc = bacc.Bacc("TRN2", target_bir_lowering=False)

    tokens = nc.declare_dram_parameter("tokens_bf", [S, D], bf16, isOutput=False)
    seg_col = nc.declare_dram_parameter("seg_col", [128, NT], f32, isOutput=False)
    seg_row = nc.declare_dram_parameter("seg_row", [1, S], f16, isOutput=False)
    iota_row = nc.declare_dram_parameter("iota_row", [128, C], f16, isOutput=False)
    iota_col = nc.declare_dram_parameter("iota_col", [128, CT], f32, isOutput=False)
    inv_cnt = nc.declare_dram_parameter("inv_cnt", [128, CT], f32, isOutput=False)
    wqkvT = nc.declare_dram_parameter("wqkvT", [L, D, 3 * D], bf16, isOutput=False)
    woT = nc.declare_dram_parameter("woT", [L, D, D], bf16, isOutput=False)
    w1T = nc.declare_dram_parameter("w1T", [L, D, DFF], bf16, isOutput=False)
    w2T = nc.declare_dram_parameter("w2T", [L, DFF, D], bf16, isOutput=False)
    if has_qkv_b:
        bqkv_c = nc.declare_dram_parameter("bqkv_c", [L, 128, 12], f32, isOutput=False)
        vb_row = nc.declare_dram_parameter("vb_row", [L, 1, D], f32, isOutput=False)
    if has_ff1_b:
        b1_c = nc.declare_dram_parameter("b1_c", [L, 128, FT], f32, isOutput=False)
    if has_out_b:
        outb_row = nc.declare_dram_parameter("outb_row", [L, 1, D], f32, isOutput=False)
    if has_ff2_b:
        ff2b_row = nc.declare_dram_parameter("ff2b_row", [L, 1, D], f32, isOutput=False)
    if has_ln1:
        ln1w_row = nc.declare_dram_parameter("ln1w_row", [L, 1, D], f32, isOutput=False)
        ln1b_row = nc.declare_dram_parameter("ln1b_row", [L, 1, D], f32, isOutput=False)
    if has_ln2:
        ln2w_row = nc.declare_dram_parameter("ln2w_row", [L, 1, D], f32, isOutput=False)
        ln2b_row = nc.declare_dram_parameter("ln2b_row", [L, 1, D], f32, isOutput=False)
    if has_fln:
        flnw_row = nc.declare_dram_parameter("flnw_row", [1, D], f32, isOutput=False)
        flnb_row = nc.declare_dram_parameter("flnb_row", [1, D], f32, isOutput=False)
    out_d = nc.declare_dram_parameter("out_bf", [S, D], bf16, isOutput=True)

    def bcast_load(pool, dram_row, tag):
        """DMA a [1, D] DRAM row into a [128, D] SBUF tile (partition bcast)."""
        t = pool.tile([128, D], f32, tag=tag, name=f"row_{tag}")
        src = bass.AP(tensor=dram_row.tensor, offset=dram_row.offset,
                      ap=[[0, 128]] + [list(p) for p in dram_row.ap[1:]])
        nc.gpsimd.dma_start(out=t, in_=src)
        return t

    with tile.TileContext(nc) as tc:
        with (
            tc.tile_pool(name="consts", bufs=1) as consts,
            tc.tile_pool(name="acts", bufs=1) as acts,
            tc.tile_pool(name="xm", bufs=2) as xmp,
            tc.tile_pool(name="xt", bufs=2) as xtp,
            tc.tile_pool(name="lnp", bufs=2) as lnp,
            tc.tile_pool(name="nrm", bufs=1) as nrm,
            tc.tile_pool(name="rows", bufs=1) as rows,
        ):
            # ---------------- constants ----------------
            seg_col_sb = consts.tile([128, NT], f32)
            nc.gpsimd.dma_start(out=seg_col_sb, in_=seg_col[:, :])
            iota_row_sb = consts.tile([128, C], f16)
            nc.gpsimd.dma_start(out=iota_row_sb, in_=iota_row[:, :])
            iota_col_sb = consts.tile([128, CT], f32)
            nc.gpsimd.dma_start(out=iota_col_sb, in_=iota_col[:, :])
            inv_cnt_sb = consts.tile([128, CT], f32)
            nc.gpsimd.dma_start(out=inv_cnt_sb, in_=inv_cnt[:, :])
            ones64b = consts.tile([1, 64], bf16)
            nc.vector.memset(ones64b, 1.0)
            ones_row32 = consts.tile([1, 128], f32)
            nc.vector.memset(ones_row32, 1.0)
            ident32 = consts.tile([128, 128], f32)
            make_identity(nc, ident32)
            eps_t = consts.tile([128, 1], f32)
            nc.vector.memset(eps_t, EPS)

            # y_bf lives in the persistent pool (used by the expand phase
            # after the weight pool is closed).
            y_bf = acts.tile([128, CT, D], bf16, tag="y_bf")

            def ln_block(ps_src, resid, wrow, brow, out_ap, pre_scale=None):
                # out = LN(ps_src * pre_scale + resid) [* w + b]  (token-major)
                t_ = lnp.tile([128, D], f32, tag="ln_t", name="ln_t")
                if resid is not None and pre_scale is not None:
                    nc.vector.scalar_tensor_tensor(
                        out=t_, in0=ps_src, scalar=pre_scale, in1=resid,
                        op0=AL.mult, op1=AL.add)
                elif resid is not None:
                    nc.vector.tensor_tensor(out=t_, in0=ps_src, in1=resid, op=AL.add)
                else:
                    nc.vector.tensor_copy(t_, ps_src)
                st = lnp.tile([128, 6], f32, tag="ln_st", name="ln_st")
                nc.vector.bn_stats(out=st, in_=t_)
                mv = lnp.tile([128, 2], f32, tag="ln_mv", name="ln_mv")
                nc.vector.bn_aggr(out=mv, in_=st)
                sd = lnp.tile([128, 1], f32, tag="ln_sd", name="ln_sd")
                nc.scalar.activation(out=sd, in_=mv[:, 1:2], func=AF.Sqrt,
                                     bias=eps_t[:, 0:1], scale=1.0)
                rs = lnp.tile([128, 1], f32, tag="ln_rs", name="ln_rs")
                nc.vector.reciprocal(rs, sd)
                if wrow is None:
                    nc.vector.tensor_scalar(
                        out=out_ap, in0=t_, scalar1=mv[:, 0:1], scalar2=rs[:, 0:1],
                        op0=AL.subtract, op1=AL.mult)
                else:
                    xn = lnp.tile([128, D], f32, tag="ln_xn", name="ln_xn")
                    nc.vector.tensor_scalar(
                        out=xn, in0=t_, scalar1=mv[:, 0:1], scalar2=rs[:, 0:1],
                        op0=AL.subtract, op1=AL.mult)
                    nc.vector.tensor_tensor(out=xn, in0=xn, in1=wrow, op=AL.mult)
                    nc.vector.tensor_tensor(out=out_ap, in0=xn, in1=brow, op=AL.add)

            # ============ scope: weights + segsum + transformer ============
            with (
                tc.tile_pool(name="wts", bufs=1) as wts,
                tc.tile_pool(name="expp", bufs=2) as expp,
            ):
                # ---- weights (bf16 in DRAM already) ----
                # All big DMAs share the ONE sync HWDGE ring so ring order =
                # transfer order: an up-front 12MB weight prefetch on its own
                # queue starves the phase-1 token stream (SDMA round-robins
                # between queues; measured: first segsum matmul at 35us).
                # Weight loads are emitted mid-phase-1 instead (see below).
                wqkv_sb = [wts.tile([128, DT, 3 * D], bf16, tag=f"wqkv{l}",
                                    name=f"wqkv{l}") for l in range(L)]
                wo_sb = [wts.tile([128, DT, D], bf16, tag=f"wo{l}",
                                  name=f"wo{l}") for l in range(L)]
                w1_sb = [wts.tile([128, DT, DFF], bf16, tag=f"w1{l}",
                                  name=f"w1{l}") for l in range(L)]
                w2_sb = [wts.tile([128, FT, D], bf16, tag=f"w2{l}",
                                  name=f"w2{l}") for l in range(L)]

                def load_weights(l):
                    nc.sync.dma_start(
                        out=wqkv_sb[l],
                        in_=wqkvT[l].rearrange("(dt p) e -> p dt e", p=128))
                    nc.sync.dma_start(
                        out=wo_sb[l],
                        in_=woT[l].rearrange("(dt p) e -> p dt e", p=128))
                    nc.sync.dma_start(
                        out=w1_sb[l],
                        in_=w1T[l].rearrange("(dt p) e -> p dt e", p=128))
                    nc.sync.dma_start(
                        out=w2_sb[l],
                        in_=w2T[l].rearrange("(ft p) e -> p ft e", p=128))

                bqkv_sb, b1_sb = [], []
                vb_sb, outb_sb, ff2b_sb = [], [], []
                ln1w_sb, ln1b_sb, ln2w_sb, ln2b_sb = [], [], [], []
                for l in range(L):
                    if has_qkv_b:
                        bq = consts.tile([128, 12], f32, tag=f"bqkv{l}", name=f"bqkv{l}")
                        nc.sync.dma_start(out=bq, in_=bqkv_c[l])
                        bqkv_sb.append(bq)
                        vb_sb.append(bcast_load(rows, vb_row[l], f"vb{l}"))
                    if has_ff1_b:
                        b1 = consts.tile([128, FT], f32, tag=f"b1{l}", name=f"b1{l}")
                        nc.sync.dma_start(out=b1, in_=b1_c[l])
                        b1_sb.append(b1)
                    if has_out_b:
                        outb_sb.append(bcast_load(rows, outb_row[l], f"outb{l}"))
                    if has_ff2_b:
                        ff2b_sb.append(bcast_load(rows, ff2b_row[l], f"ff2b{l}"))
                    if has_ln1:
                        ln1w_sb.append(bcast_load(rows, ln1w_row[l], f"ln1w{l}"))
                        ln1b_sb.append(bcast_load(rows, ln1b_row[l], f"ln1b{l}"))
                    if has_ln2:
                        ln2w_sb.append(bcast_load(rows, ln2w_row[l], f"ln2w{l}"))
                        ln2b_sb.append(bcast_load(rows, ln2b_row[l], f"ln2b{l}"))
                flnw_sb = bcast_load(rows, flnw_row, "flnw") if has_fln else None
                flnb_sb = bcast_load(rows, flnb_row, "flnb") if has_fln else None

                # ------------ phase 1: segment sums -> means ------------
                # bf16 token tiles stream over HWDGE in 1 MiB groups of 8;
                # one-hot matmuls accumulate sums in PSUM; host-computed
                # 1/counts turns them into means (no count matmuls).
                x0 = xmp.tile([128, CT, D], f32, tag="xm", name="x0")
                with (
                    tc.tile_pool(name="pseg", bufs=1, space="PSUM") as pseg,
                    tc.tile_pool(name="segs", bufs=3) as segs,
                    tc.tile_pool(name="ohp1", bufs=3) as ohp1,
                ):
                    ps_sums = [pseg.tile([128, D], f32, tag=f"sums{m}", name=f"sums{m}")
                               for m in range(CT)]
                    for g in range(NG):
                        tokg = segs.tile([128, GT, D], bf16, tag="tokg", name="tokg")
                        nc.sync.dma_start(
                            out=tokg,
                            in_=tokens[g * GT * 128:(g + 1) * GT * 128, :]
                            .rearrange("(n p) d -> p n d", p=128))
                        for j in range(GT):
                            t = g * GT + j
                            lo, hi = ranges[t]
                            oh = ohp1.tile([128, C], bf16, tag="oh", name="oh")
                            sl = slice(lo * 128, (hi + 1) * 128)
                            nc.vector.tensor_scalar(
                                out=oh[:, sl], in0=iota_row_sb[:, sl],
                                scalar1=seg_col_sb[:, t:t + 1],
                                scalar2=None, op0=AL.is_equal)
                            for m in range(lo, hi + 1):
                                nc.tensor.matmul(
                                    ps_sums[m], lhsT=oh[:, m * 128:(m + 1) * 128],
                                    rhs=tokg[:, j, :],
                                    start=(t == first_t[m]), stop=(t == last_t[m]))
                        if g == NG - 5:
                            # Ghost WAW dep: the tiny copy into the weight
                            # tile pins the qkv0 DMA AFTER group-3 tokens in
                            # the scheduler (emission order alone is just a
                            # priority hint and gets hoisted).
                            nc.vector.tensor_copy(wqkv_sb[0][0:1, 0, 0:1],
                                                  tokg[0:1, 0, 0:1])
                            nc.sync.dma_start(
                                out=wqkv_sb[0],
                                in_=wqkvT[0].rearrange("(dt p) e -> p dt e",
                                                       p=128))
                    # x = sums * (1/count)
                    for m in range(CT):
                        nc.vector.tensor_scalar(
                            out=x0[:, m, :], in0=ps_sums[m],
                            scalar1=inv_cnt_sb[:, m:m + 1],
                            scalar2=None, op0=AL.mult)
                    # remaining weights stream during phase-2 compute, in ring
                    # order, ghost-dep'd on x0 so they cannot be hoisted into
                    # the token stream.
                    for wtile in (wo_sb[0], w1_sb[0], wqkv_sb[1], wo_sb[1],
                                  w1_sb[1]):
                        nc.vector.tensor_copy(wtile[0:1, 0, 0:1],
                                              x0[0:1, 0, 0:1])
                    for wtile in (w2_sb[0], w2_sb[1]):
                        nc.vector.tensor_copy(wtile[0:1, 0, 0:1],
                                              x0[0:1, 0, 0:1])
                    nc.sync.dma_start(
                        out=wo_sb[0],
                        in_=woT[0].rearrange("(dt p) e -> p dt e", p=128))
                    nc.sync.dma_start(
                        out=w1_sb[0],
                        in_=w1T[0].rearrange("(dt p) e -> p dt e", p=128))
                    nc.sync.dma_start(
                        out=w2_sb[0],
                        in_=w2T[0].rearrange("(ft p) e -> p ft e", p=128))
                    load_weights(1)

                # ---------------- phase 2: transformer ----------------
                with (
                    tc.tile_pool(name="psA", bufs=2, space="PSUM") as psA,
                    tc.tile_pool(name="psS", bufs=2, space="PSUM") as psS,
                    tc.tile_pool(name="psO", bufs=2, space="PSUM") as psO,
                ):
                    def transpose_to(src_f32, dst_bf16):
                        # src: [128, CT, D] f32 token-major; dst: [128, DT, C] bf16
                        for i in range(CT):
                            for j in range(DT):
                                pst = psS.tile([128, 128], f32, tag="ps_t", name="ps_t")
                                nc.tensor.transpose(
                                    pst, src_f32[:, i, j * 128:(j + 1) * 128], ident32)
                                nc.vector.tensor_copy(
                                    dst_bf16[:, j, i * 128:(i + 1) * 128], pst)

                    x_in = x0
                    for l in range(L):
                        xT = xtp.tile([128, DT, C], bf16, tag="xT", name="xT")
                        transpose_to(x_in, xT)

                        # --- q, k feature-major [e, c] ---
                        qT = acts.tile([128, DT, C], bf16, tag="qT", name="qT")
                        kT = acts.tile([128, DT, C], bf16, tag="kT", name="kT")
                        for et in range(8):
                            ps = psA.tile([128, C], f32, tag="ps_a", name="ps_a")
                            for dt_ in range(DT):
                                nc.tensor.matmul(
                                    ps, lhsT=wqkv_sb[l][:, dt_, et * 128:(et + 1) * 128],
                                    rhs=xT[:, dt_, :],
                                    start=(dt_ == 0), stop=(dt_ == DT - 1))
                            dst = qT[:, et, :] if et < 4 else kT[:, et - 4, :]
                            if has_qkv_b:
                                nc.scalar.activation(
                                    out=dst, in_=ps, func=AF.Identity,
                                    bias=bqkv_sb[l][:, et:et + 1], scale=1.0)
                            else:
                                nc.scalar.copy(out=dst, in_=ps)

                        # --- v token-major [c, e] with per-head ones column ---
                        v_ext = acts.tile([128, CT, H, HD + 1], bf16, tag="v_ext",
                                          name="v_ext")
                        nc.vector.memset(v_ext[:, :, :, HD:HD + 1], 1.0)
                        for ct in range(CT):
                            ps = psA.tile([128, C], f32, tag="ps_a", name="ps_a")
                            for dt_ in range(DT):
                                nc.tensor.matmul(
                                    ps, lhsT=xT[:, dt_, ct * 128:(ct + 1) * 128],
                                    rhs=wqkv_sb[l][:, dt_, 2 * D:3 * D],
                                    start=(dt_ == 0), stop=(dt_ == DT - 1))
                            if has_qkv_b:
                                tv = lnp.tile([128, D], f32, tag="ln_t", name="tv")
                                nc.vector.tensor_tensor(out=tv, in0=ps, in1=vb_sb[l],
                                                        op=AL.add)
                                nc.scalar.copy(out=v_ext[:, ct, :, 0:HD], in_=tv)
                            else:
                                nc.scalar.copy(out=v_ext[:, ct, :, 0:HD], in_=ps)

                        # --- attention, software-pipelined across heads ---
                        # PE order: scores(h) ... scores(h+1), av(h), so the PE
                        # never sits behind ACT's exp in its own queue.
                        oT = acts.tile([128, DT, C], bf16, tag="oT", name="oT")
                        expTs = [None] * H     # live expT tiles per head
                        psOs = [None] * H      # live AV psum per head

                        def emit_scores(h):
                            th, off = h // 2, (h % 2) * 64
                            expT = expp.tile([128, CT, C], bf16, tag="expT",
                                             name="expT")
                            for kt in range(CT):
                                ps = psS.tile([128, C], f32, tag="ps_s", name="ps_s")
                                nc.tensor.matmul(
                                    ps,
                                    lhsT=kT[off:off + 64, th, kt * 128:(kt + 1) * 128],
                                    rhs=qT[off:off + 64, th, :], start=True, stop=True)
                                nc.scalar.activation(out=expT[:, kt, :], in_=ps,
                                                     func=AF.Exp, scale=1.0 / 8.0)
                            expTs[h] = expT

                        def emit_av(h):
                            pso = psO.tile([128, C], f32, tag="ps_o", name="ps_o")
                            for kt in range(CT):
                                nc.tensor.matmul(
                                    pso[0:HD + 1, :], lhsT=v_ext[:, kt, h, :],
                                    rhs=expTs[h][:, kt, :],
                                    start=(kt == 0), stop=(kt == CT - 1))
                            psOs[h] = pso

                        def emit_norm(p):
                            # heads 2p (rows 0:64) and 2p+1 (rows 64:128)
                            h0, h1 = 2 * p, 2 * p + 1
                            th = p
                            # denominator rows to SBUF (custom DVE ops must
                            # not read PSUM), ONE fast reciprocal, bf16 cast,
                            # 1-cyc/row bf16 broadcast matmuls into the two
                            # partition halves, single ACT bounce to SBUF,
                            # then two PSUM-direct DVE mults.
                            den2 = nrm.tile([1, 2, C], f32, tag="den2",
                                            name="den2")
                            nc.vector.tensor_copy(den2[:, 0, :],
                                                  psOs[h0][HD:HD + 1, :])
                            nc.vector.tensor_copy(den2[:, 1, :],
                                                  psOs[h1][HD:HD + 1, :])
                            rec2 = nrm.tile([1, 2, C], f32, tag="rec2",
                                            name="rec2")
                            nc.vector.reciprocal_approx_fast(rec2, den2)
                            rec2b = nrm.tile([1, 2, C], bf16, tag="rec2b",
                                             name="rec2b")
                            nc.vector.tensor_copy(rec2b, rec2)
                            psd = psA.tile([128, C], f32, tag="ps_a", name="ps_d")
                            nc.tensor.matmul(
                                psd[0:64, :], lhsT=ones64b, rhs=rec2b[:, 0, :],
                                start=True, stop=True, skip_group_check=True)
                            nc.tensor.matmul(
                                psd[64:128, :], lhsT=ones64b, rhs=rec2b[:, 1, :],
                                start=True, stop=True, skip_group_check=True)
                            rec_bc = nrm.tile([128, C], f32, tag="rec_bc",
                                              name="rec_bc")
                            nc.scalar.copy(out=rec_bc, in_=psd)
                            nc.vector.tensor_tensor(
                                out=oT[0:64, th, :], in0=psOs[h0][0:HD, :],
                                in1=rec_bc[0:64, :], op=AL.mult)
                            nc.vector.tensor_tensor(
                                out=oT[64:128, th, :], in0=psOs[h1][0:HD, :],
                                in1=rec_bc[64:128, :], op=AL.mult)
                            psOs[h0] = psOs[h1] = None

                        emit_scores(0)
                        for h in range(1, H):
                            emit_scores(h)
                            emit_av(h - 1)
                            if h >= 2 and h % 2 == 0:
                                emit_norm((h - 2) // 2)
                        emit_av(H - 1)
                        emit_norm(H // 2 - 1)

                        # --- out-projection + residual + LN1 ---
                        xm2 = xmp.tile([128, CT, D], f32, tag="xm", name="xm2")
                        for ct in range(CT):
                            ps = psA.tile([128, C], f32, tag="ps_a", name="ps_a")
                            for et in range(DT):
                                nc.tensor.matmul(
                                    ps, lhsT=oT[:, et, ct * 128:(ct + 1) * 128],
                                    rhs=wo_sb[l][:, et, :],
                                    start=(et == 0), stop=(et == DT - 1))
                            if has_out_b:
                                nc.vector.tensor_tensor(out=ps, in0=ps, in1=outb_sb[l],
                                                        op=AL.add)
                            ln_block(ps, x_in[:, ct, :],
                                     ln1w_sb[l] if has_ln1 else None,
                                     ln1b_sb[l] if has_ln1 else None,
                                     xm2[:, ct, :])

                        # --- FFN ---
                        x2T = xtp.tile([128, DT, C], bf16, tag="xT", name="x2T")
                        transpose_to(xm2, x2T)
                        hT = acts.tile([128, FT, C], bf16, tag="hT", name="hT")
                        for ft in range(FT):
                            ps = psA.tile([128, C], f32, tag="ps_a", name="ps_a")
                            for dt_ in range(DT):
                                nc.tensor.matmul(
                                    ps, lhsT=w1_sb[l][:, dt_, ft * 128:(ft + 1) * 128],
                                    rhs=x2T[:, dt_, :],
                                    start=(dt_ == 0), stop=(dt_ == DT - 1))
                            nc.scalar.activation(
                                out=hT[:, ft, :], in_=ps, func=AF.Relu,
                                bias=(b1_sb[l][:, ft:ft + 1] if has_ff1_b else 0.0),
                                scale=1.0)
                        x_next = xmp.tile([128, CT, D], f32, tag="xm", name="x_next")
                        for ct in range(CT):
                            ps = psA.tile([128, C], f32, tag="ps_a", name="ps_a")
                            for ft in range(FT):
                                nc.tensor.matmul(
                                    ps, lhsT=hT[:, ft, ct * 128:(ct + 1) * 128],
                                    rhs=w2_sb[l][:, ft, :],
                                    start=(ft == 0), stop=(ft == FT - 1))
                            if has_ff2_b:
                                nc.vector.tensor_tensor(out=ps, in0=ps, in1=ff2b_sb[l],
                                                        op=AL.add)
                            ln_block(ps, xm2[:, ct, :],
                                     ln2w_sb[l] if has_ln2 else None,
                                     ln2b_sb[l] if has_ln2 else None,
                                     x_next[:, ct, :])
                        x_in = x_next

                    # ---------------- phase 3: final LN -> y_bf ----------------
                    for ct in range(CT):
                        ln_block(x_in[:, ct, :], None, flnw_sb, flnb_sb,
                                 y_bf[:, ct, :])

            # ============ scope: expand ============
            # out[s, :] = y[seg[s], :] via one-hot^T matmuls; bf16 output in
            # 1 MiB group DMAs.
            with (
                tc.tile_pool(name="ohp", bufs=3) as ohp,
                tc.tile_pool(name="outp", bufs=2) as outp,
                tc.tile_pool(name="psE", bufs=4, space="PSUM") as psE,
            ):
                seg_row_ap = seg_row[:, :]
                for g in range(NG):
                    seg_bc = ohp.tile([128, GT * 128], f16, tag="seg_bc",
                                      name="seg_bc")
                    src = bass.AP(tensor=seg_row_ap.tensor, offset=g * GT * 128,
                                  ap=[[0, 128], [1, GT * 128]])
                    nc.gpsimd.dma_start(out=seg_bc, in_=src)
                    og = outp.tile([128, GT, D], bf16, tag="og", name="og")
                    for j in range(GT):
                        t = g * GT + j
                        lo, hi = ranges[t]
                        ohT = ohp.tile([128, CT, 128], bf16, tag="ohT", name="ohT")
                        for m in range(lo, hi + 1):
                            nc.vector.tensor_scalar(
                                out=ohT[:, m, :],
                                in0=seg_bc[:, j * 128:(j + 1) * 128],
                                scalar1=iota_col_sb[:, m:m + 1], scalar2=None,
                                op0=AL.is_equal)
                        pse = psE.tile([128, D], f32, tag="ps_e", name="ps_e")
                        for m in range(lo, hi + 1):
                            nc.tensor.matmul(
                                pse, lhsT=ohT[:, m, :],
                                rhs=y_bf[:, m, :],
                                start=(m == lo), stop=(m == hi))
                        if t % 2 == 0:
                            nc.scalar.copy(out=og[:, j, :], in_=pse)
                        else:
                            nc.vector.tensor_copy(og[:, j, :], pse)
                    nc.sync.dma_start(
                        out=out_d[g * GT * 128:(g + 1) * GT * 128, :]
                        .rearrange("(n p) d -> p n d", p=128),
                        in_=og)

    return nc


def _host_prep(inputs):
    """Shard + preprocess full inputs into 8 per-core input maps."""
    bf = ml_dtypes.bfloat16
    f8 = ml_dtypes.float8_e4m3fn
    tokens = np.asarray(inputs["tokens"], dtype=np.float32)
    seg = np.asarray(inputs["segment_ids"], dtype=np.int32)
    qkv_w = np.asarray(inputs["qkv_w"], dtype=np.float32)
    qkv_b = np.asarray(inputs["qkv_b"], dtype=np.float32)
    out_w = np.asarray(inputs["out_w"], dtype=np.float32)
    out_b = np.asarray(inputs["out_b"], dtype=np.float32)
    ln1_w = np.asarray(inputs["ln1_w"], dtype=np.float32)
    ln1_b = np.asarray(inputs["ln1_b"], dtype=np.float32)
    ln2_w = np.asarray(inputs["ln2_w"], dtype=np.float32)
    ln2_b = np.asarray(inputs["ln2_b"], dtype=np.float32)
    ff1_w = np.asarray(inputs["ff1_w"], dtype=np.float32)
    ff1_b = np.asarray(inputs["ff1_b"], dtype=np.float32)
    ff2_w = np.asarray(inputs["ff2_w"], dtype=np.float32)
    ff2_b = np.asarray(inputs["ff2_b"], dtype=np.float32)
    fln_w = np.asarray(inputs["fln_w"], dtype=np.float32)
    fln_b = np.asarray(inputs["fln_b"], dtype=np.float32)

    flags = (
        bool(np.any(qkv_b)),
        bool(np.any(out_b)),
        bool(np.any(ff1_b)),
        bool(np.any(ff2_b)),
        bool(np.any(ln1_w != 1.0) or np.any(ln1_b)),
        bool(np.any(ln2_w != 1.0) or np.any(ln2_b)),
        bool(np.any(fln_w != 1.0) or np.any(fln_b)),
    )

    # span-bound ranges: per token tile, union over batch of the contiguous
    # chunk-tile range its (sorted) segment ids cover.
    srt = np.all(np.diff(seg, axis=1) >= 0)
    if srt:
        lo = np.min(seg[:, ::128] // 128, axis=0)
        hi = np.max(seg[:, 127::128] // 128, axis=0)
    else:  # fallback: no structure assumed
        lo = np.zeros(NT, np.int64)
        hi = np.full(NT, CT - 1, np.int64)
    covered = set()
    for t in range(NT):
        covered.update(range(int(lo[t]), int(hi[t]) + 1))
    if covered != set(range(CT)):
        lo = np.zeros(NT, np.int64)
        hi = np.full(NT, CT - 1, np.int64)
    ranges = tuple((int(lo[t]), int(hi[t])) for t in range(NT))

    # shared (batch-independent) arrays
    shared = {
        "iota_row": np.broadcast_to(
            np.arange(C, dtype=np.float16)[None, :], (128, C)).copy(),
        "iota_col": (np.arange(CT, dtype=np.float32)[None, :] * 128
                     + np.arange(128, dtype=np.float32)[:, None]).astype(np.float32),
        "wqkvT": np.ascontiguousarray(qkv_w.transpose(0, 2, 1)).astype(bf),
        "woT": np.ascontiguousarray(out_w.transpose(0, 2, 1)).astype(bf),
        "w1T": np.ascontiguousarray(ff1_w.transpose(0, 2, 1)).astype(bf),
        "w2T": np.ascontiguousarray(ff2_w.transpose(0, 2, 1)).astype(bf),
    }
    (has_qkv_b, has_out_b, has_ff1_b, has_ff2_b,
     has_ln1, has_ln2, has_fln) = flags
    if has_qkv_b:
        shared["bqkv_c"] = np.ascontiguousarray(
            qkv_b[:, :1536].reshape(L, 12, 128).transpose(0, 2, 1))
        shared["vb_row"] = np.ascontiguousarray(qkv_b[:, 2 * D:3 * D][:, None, :])
    if has_ff1_b:
        shared["b1_c"] = np.ascontiguousarray(
            ff1_b.reshape(L, FT, 128).transpose(0, 2, 1))
    if has_out_b:
        shared["outb_row"] = np.ascontiguousarray(out_b[:, None, :])
    if has_ff2_b:
        shared["ff2b_row"] = np.ascontiguousarray(ff2_b[:, None, :])
    if has_ln1:
        shared["ln1w_row"] = np.ascontiguousarray(ln1_w[:, None, :])
        shared["ln1b_row"] = np.ascontiguousarray(ln1_b[:, None, :])
    if has_ln2:
        shared["ln2w_row"] = np.ascontiguousarray(ln2_w[:, None, :])
        shared["ln2b_row"] = np.ascontiguousarray(ln2_b[:, None, :])
    if has_fln:
        shared["flnw_row"] = np.ascontiguousarray(fln_w[None, :])
        shared["flnb_row"] = np.ascontiguousarray(fln_b[None, :])

    in_maps = []
    for b in range(B):
        m = dict(shared)
        m["tokens_bf"] = np.ascontiguousarray(tokens[b]).astype(bf)
        m["seg_col"] = np.ascontiguousarray(
            seg[b].reshape(NT, 128).T.astype(np.float32))
        m["seg_row"] = np.ascontiguousarray(seg[b].astype(np.float16)[None, :])
        counts = np.bincount(seg[b], minlength=C).astype(np.float32)
        m["inv_cnt"] = np.ascontiguousarray(
            (1.0 / np.maximum(counts, 1.0)).reshape(CT, 128).T)
        in_maps.append(m)
    return flags, ranges, in_maps


def kernel(**inputs) -> np.ndarray:
    from concourse.bass_utils import run_bass_kernel_spmd

    flags, ranges, in_maps = _host_prep(inputs)
    key = (flags, ranges)
    if key not in _CACHE:
        nc = _build(flags, ranges)
        if not nc.is_finalized():
            nc.finalize()
        _CACHE[key] = nc
    nc = _CACHE[key]
    res = run_bass_kernel_spmd(nc, in_maps, list(range(B)))
    return np.stack(
        [np.asarray(res.results[i]["out_bf"]).astype(np.float32)
         for i in range(B)], axis=0)


# revision 34
# speedup vs baseline: 1.0428x; 1.0428x over previous
"""Trainium2 Bass kernel for nn_ChunkProcessor (segment-mean -> 2-layer
transformer encoder over chunks -> gather-expand -> final LN).

Sharding: data-parallel over batch B=8 across the 8 NeuronCores; each core
processes one batch item end to end (no cross-core communication).

v2 design notes (perf):
  - tokens / weights pre-cast to bf16 on HOST; output written bf16 and
    upcast on host: halves all big HBM traffic.
  - 1/counts computed on host -> no count matmuls, no phase-1 reciprocals,
    PSUM banks freed.
  - tokens + output move in 1 MiB group DMAs (8 tiles each) for full DMA BW.
  - attention softmax normalization: denominators from the v ones-column,
    reciprocal_approx_fast on a [2,C] pair tile, broadcast to 128 partitions
    with ONE fp32r matmul per head pair (1 cyc/row), numerators bounced to
    SBUF on ACT, one DVE mult per head. Kills the [1,C] exact reciprocals
    (1.4us each) and fp32 1x64 broadcast matmuls (1us each) of v1.
  - attention software-pipelined: scores for head h+1 are emitted on PE
    before AV of head h so the PE never waits on ACT's exp -- keeps the PE
    HAM clock gate at 2.4 GHz (idle PE re-throttles to 1.2 GHz).
"""

import numpy as np
import ml_dtypes

B, S, D = 8, 8192, 512
C, H, L, DFF = 512, 8, 2, 2048
HD = D // H          # 64
NT = S // 128        # 64 token tiles
CT = C // 128        # 4 chunk tiles
DT = D // 128        # 4 feature tiles
FT = DFF // 128      # 16
GT = 4               # token tiles per DMA group
NG = NT // GT        # token-tile DMA groups
SW = 64.0            # fp8 weight pre-scale (host); folded back as 1/SW
EPS = 1e-5

_CACHE = {}


def _build(flags, ranges):
    """Build the Bass program.

    flags  = (qkv_b, out_b, ff1_b, ff2_b, ln1_aff, ln2_aff, fln_aff) bools.
    ranges = tuple of (lo_m, hi_m) per token tile t: the contiguous range of
             chunk tiles any batch item's tile-t segment ids fall into.
    """
    import concourse.bass as bass
    import concourse.tile as tile
    from concourse import bacc, mybir
    from concourse.masks import make_identity

    (has_qkv_b, has_out_b, has_ff1_b, has_ff2_b,
     has_ln1, has_ln2, has_fln) = flags

    # first/last contributing token tile per chunk tile (for PSUM start/stop)
    first_t = [min(t for t in range(NT) if ranges[t][0] <= m <= ranges[t][1])
               for m in range(CT)]
    last_t = [max(t for t in range(NT) if ranges[t][0] <= m <= ranges[t][1])
              for m in range(CT)]

    f32 = mybir.dt.float32
    f32r = mybir.dt.float32r
    bf16 = mybir.dt.bfloat16
    f16 = mybir.dt.float16
    fp8 = mybir.dt.float8e4
    AL = mybir.AluOpType
    AF = mybir.ActivationFunctionType
    DR = mybir.MatmulPerfMode.DoubleRow

    nc = bacc.Bacc("TRN2", target_bir_lowering=False)

    tokens = nc.declare_dram_parameter("tokens_bf", [S, D], bf16, isOutput=False)
    seg_col = nc.declare_dram_parameter("seg_col", [128, NT], f32, isOutput=False)
    seg_row = nc.declare_dram_parameter("seg_row", [1, S], f16, isOutput=False)
    iota_row = nc.declare_dram_parameter("iota_row", [128, C], f16, isOutput=False)
    iota_col = nc.declare_dram_parameter("iota_col", [128, CT], f32, isOutput=False)
    inv_cnt = nc.declare_dram_parameter("inv_cnt", [128, CT], f32, isOutput=False)
    wqkvT = nc.declare_dram_parameter("wqkvT", [L, D, 3 * D], bf16, isOutput=False)
    woT = nc.declare_dram_parameter("woT", [L, D, D], bf16, isOutput=False)
    w1T = nc.declare_dram_parameter("w1T", [L, D, DFF], bf16, isOutput=False)
    w2T = nc.declare_dram_parameter("w2T", [L, DFF, D], bf16, isOutput=False)
    if has_qkv_b:
        bqkv_c = nc.declare_dram_parameter("bqkv_c", [L, 128, 12], f32, isOutput=False)
        vb_row = nc.declare_dram_parameter("vb_row", [L, 1, D], f32, isOutput=False)
    if has_ff1_b:
        b1_c = nc.declare_dram_parameter("b1_c", [L, 128, FT], f32, isOutput=False)
    if has_out_b:
        outb_row = nc.declare_dram_parameter("outb_row", [L, 1, D], f32, isOutput=False)
    if has_ff2_b:
        ff2b_row = nc.declare_dram_parameter("ff2b_row", [L, 1, D], f32, isOutput=False)
    if has_ln1:
        ln1w_row = nc.declare_dram_parameter("ln1w_row", [L, 1, D], f32, isOutput=False)
        ln1b_row = nc.declare_dram_parameter("ln1b_row", [L, 1, D], f32, isOutput=False)
    if has_ln2:
        ln2w_row = nc.declare_dram_parameter("ln2w_row", [L, 1, D], f32, isOutput=False)
        ln2b_row = nc.declare_dram_parameter("ln2b_row", [L, 1, D], f32, isOutput=False)
    if has_fln:
        flnw_row = nc.declare_dram_parameter("flnw_row", [1, D], f32, isOutput=False)
        flnb_row = nc.declare_dram_parameter("flnb_row", [1, D], f32, isOutput=False)
    out_d = nc.declare_dram_parameter("out_bf", [S, D], bf16, isOutput=True)

    def bcast_load(pool, dram_row, tag):
        """DMA a [1, D] DRAM row into a [128, D] SBUF tile (partition bcast)."""
        t = pool.tile([128, D], f32, tag=tag, name=f"row_{tag}")
        src = bass.AP(tensor=dram_row.tensor, offset=dram_row.offset,
                      ap=[[0, 128]] + [list(p) for p in dram_row.ap[1:]])
        nc.gpsimd.dma_start(out=t, in_=src)
        return t

    with tile.TileContext(nc) as tc:
        with (
            tc.tile_pool(name="consts", bufs=1) as consts,
            tc.tile_pool(name="acts", bufs=1) as acts,
            tc.tile_pool(name="xm", bufs=2) as xmp,
            tc.tile_pool(name="xt", bufs=2) as xtp,
            tc.tile_pool(name="lnp", bufs=2) as lnp,
            tc.tile_pool(name="nrm", bufs=1) as nrm,
            tc.tile_pool(name="rows", bufs=1) as rows,
        ):
            # ---------------- constants ----------------
            seg_col_sb = consts.tile([128, NT], f32)
            nc.gpsimd.dma_start(out=seg_col_sb, in_=seg_col[:, :])
            iota_row_sb = consts.tile([128, C], f16)
            nc.gpsimd.dma_start(out=iota_row_sb, in_=iota_row[:, :])
            iota_col_sb = consts.tile([128, CT], f32)
            nc.gpsimd.dma_start(out=iota_col_sb, in_=iota_col[:, :])
            inv_cnt_sb = consts.tile([128, CT], f32)
            nc.gpsimd.dma_start(out=inv_cnt_sb, in_=inv_cnt[:, :])
            ones64b = consts.tile([1, 64], bf16)
            nc.vector.memset(ones64b, 1.0)
            ones_row32 = consts.tile([1, 128], f32)
            nc.vector.memset(ones_row32, 1.0)
            ident32 = consts.tile([128, 128], f32)
            make_identity(nc, ident32)
            eps_t = consts.tile([128, 1], f32)
            nc.vector.memset(eps_t, EPS)

            # y_bf lives in the persistent pool (used by the expand phase
            # after the weight pool is closed).
            y_bf = acts.tile([128, CT, D], bf16, tag="y_bf")

            def ln_block(ps_src, resid, wrow, brow, out_ap, pre_scale=None):
                # out = LN(ps_src * pre_scale + resid) [* w + b]  (token-major)
                t_ = lnp.tile([128, D], f32, tag="ln_t", name="ln_t")
                if resid is not None and pre_scale is not None:
                    nc.vector.scalar_tensor_tensor(
                        out=t_, in0=ps_src, scalar=pre_scale, in1=resid,
                        op0=AL.mult, op1=AL.add)
                elif resid is not None:
                    nc.vector.tensor_tensor(out=t_, in0=ps_src, in1=resid, op=AL.add)
                else:
                    nc.vector.tensor_copy(t_, ps_src)
                st = lnp.tile([128, 6], f32, tag="ln_st", name="ln_st")
                nc.vector.bn_stats(out=st, in_=t_)
                mv = lnp.tile([128, 2], f32, tag="ln_mv", name="ln_mv")
                nc.vector.bn_aggr(out=mv, in_=st)
                sd = lnp.tile([128, 1], f32, tag="ln_sd", name="ln_sd")
                nc.scalar.activation(out=sd, in_=mv[:, 1:2], func=AF.Sqrt,
                                     bias=eps_t[:, 0:1], scale=1.0)
                rs = lnp.tile([128, 1], f32, tag="ln_rs", name="ln_rs")
                nc.vector.reciprocal(rs, sd)
                if wrow is None:
                    nc.vector.tensor_scalar(
                        out=out_ap, in0=t_, scalar1=mv[:, 0:1], scalar2=rs[:, 0:1],
                        op0=AL.subtract, op1=AL.mult)
                else:
                    xn = lnp.tile([128, D], f32, tag="ln_xn", name="ln_xn")
                    nc.vector.tensor_scalar(
                        out=xn, in0=t_, scalar1=mv[:, 0:1], scalar2=rs[:, 0:1],
                        op0=AL.subtract, op1=AL.mult)
                    nc.vector.tensor_tensor(out=xn, in0=xn, in1=wrow, op=AL.mult)
                    nc.vector.tensor_tensor(out=out_ap, in0=xn, in1=brow, op=AL.add)

            # ============ scope: weights + segsum + transformer ============
            with (
                tc.tile_pool(name="wts", bufs=1) as wts,
                tc.tile_pool(name="expp", bufs=2) as expp,
            ):
                # ---- weights (bf16 in DRAM already) ----
                # All big DMAs share the ONE sync HWDGE ring so ring order =
                # transfer order: an up-front 12MB weight prefetch on its own
                # queue starves the phase-1 token stream (SDMA round-robins
                # between queues; measured: first segsum matmul at 35us).
                # Weight loads are emitted mid-phase-1 instead (see below).
                wqkv_sb = [wts.tile([128, DT, 3 * D], bf16, tag=f"wqkv{l}",
                                    name=f"wqkv{l}") for l in range(L)]
                wo_sb = [wts.tile([128, DT, D], bf16, tag=f"wo{l}",
                                  name=f"wo{l}") for l in range(L)]
                w1_sb = [wts.tile([128, DT, DFF], bf16, tag=f"w1{l}",
                                  name=f"w1{l}") for l in range(L)]
                w2_sb = [wts.tile([128, FT, D], bf16, tag=f"w2{l}",
                                  name=f"w2{l}") for l in range(L)]

                def load_weights(l):
                    nc.sync.dma_start(
                        out=wqkv_sb[l],
                        in_=wqkvT[l].rearrange("(dt p) e -> p dt e", p=128))
                    nc.sync.dma_start(
                        out=wo_sb[l],
                        in_=woT[l].rearrange("(dt p) e -> p dt e", p=128))
                    nc.sync.dma_start(
                        out=w1_sb[l],
                        in_=w1T[l].rearrange("(dt p) e -> p dt e", p=128))
                    nc.sync.dma_start(
                        out=w2_sb[l],
                        in_=w2T[l].rearrange("(ft p) e -> p ft e", p=128))

                bqkv_sb, b1_sb = [], []
                vb_sb, outb_sb, ff2b_sb = [], [], []
                ln1w_sb, ln1b_sb, ln2w_sb, ln2b_sb = [], [], [], []
                for l in range(L):
                    if has_qkv_b:
                        bq = consts.tile([128, 12], f32, tag=f"bqkv{l}", name=f"bqkv{l}")
                        nc.sync.dma_start(out=bq, in_=bqkv_c[l])
                        bqkv_sb.append(bq)
                        vb_sb.append(bcast_load(rows, vb_row[l], f"vb{l}"))
                    if has_ff1_b:
                        b1 = consts.tile([128, FT], f32, tag=f"b1{l}", name=f"b1{l}")
                        nc.sync.dma_start(out=b1, in_=b1_c[l])
                        b1_sb.append(b1)
                    if has_out_b:
                        outb_sb.append(bcast_load(rows, outb_row[l], f"outb{l}"))
                    if has_ff2_b:
                        ff2b_sb.append(bcast_load(rows, ff2b_row[l], f"ff2b{l}"))
                    if has_ln1:
                        ln1w_sb.append(bcast_load(rows, ln1w_row[l], f"ln1w{l}"))
                        ln1b_sb.append(bcast_load(rows, ln1b_row[l], f"ln1b{l}"))
                    if has_ln2:
                        ln2w_sb.append(bcast_load(rows, ln2w_row[l], f"ln2w{l}"))
                        ln2b_sb.append(bcast_load(rows, ln2b_row[l], f"ln2b{l}"))
                flnw_sb = bcast_load(rows, flnw_row, "flnw") if has_fln else None
                flnb_sb = bcast_load(rows, flnb_row, "flnb") if has_fln else None

                # ------------ phase 1: segment sums -> means ------------
                # bf16 token tiles stream over HWDGE in 1 MiB groups of 8;
                # one-hot matmuls accumulate sums in PSUM; host-computed
                # 1/counts turns them into means (no count matmuls).
                x0 = xmp.tile([128, CT, D], f32, tag="xm", name="x0")
                with (
                    tc.tile_pool(name="pseg", bufs=1, space="PSUM") as pseg,
                    tc.tile_pool(name="segs", bufs=3) as segs,
                    tc.tile_pool(name="ohp1", bufs=3) as ohp1,
                ):
                    ps_sums = [pseg.tile([128, D], f32, tag=f"sums{m}", name=f"sums{m}")
                               for m in range(CT)]
                    for g in range(NG):
                        tokg = segs.tile([128, GT, D], bf16, tag="tokg", name="tokg")
                        nc.sync.dma_start(
                            out=tokg,
                            in_=tokens[g * GT * 128:(g + 1) * GT * 128, :]
                            .rearrange("(n p) d -> p n d", p=128))
                        for j in range(GT):
                            t = g * GT + j
                            lo, hi = ranges[t]
                            oh = ohp1.tile([128, C], bf16, tag="oh", name="oh")
                            sl = slice(lo * 128, (hi + 1) * 128)
                            nc.vector.tensor_scalar(
                                out=oh[:, sl], in0=iota_row_sb[:, sl],
                                scalar1=seg_col_sb[:, t:t + 1],
                                scalar2=None, op0=AL.is_equal)
                            for m in range(lo, hi + 1):
                                nc.tensor.matmul(
                                    ps_sums[m], lhsT=oh[:, m * 128:(m + 1) * 128],
                                    rhs=tokg[:, j, :],
                                    start=(t == first_t[m]), stop=(t == last_t[m]))
                        if g == NG - 5:
                            # Ghost WAW dep: the tiny copy into the weight
                            # tile pins the qkv0 DMA AFTER group-3 tokens in
                            # the scheduler (emission order alone is just a
                            # priority hint and gets hoisted).
                            nc.vector.tensor_copy(wqkv_sb[0][0:1, 0, 0:1],
                                                  tokg[0:1, 0, 0:1])
                            nc.sync.dma_start(
                                out=wqkv_sb[0],
                                in_=wqkvT[0].rearrange("(dt p) e -> p dt e",
                                                       p=128))
                    # x = sums * (1/count)
                    for m in range(CT):
                        nc.vector.tensor_scalar(
                            out=x0[:, m, :], in0=ps_sums[m],
                            scalar1=inv_cnt_sb[:, m:m + 1],
                            scalar2=None, op0=AL.mult)
                    # remaining weights stream during phase-2 compute, in ring
                    # order, ghost-dep'd on x0 so they cannot be hoisted into
                    # the token stream.
                    for wtile in (wo_sb[0], w1_sb[0], wqkv_sb[1], wo_sb[1],
                                  w1_sb[1]):
                        nc.vector.tensor_copy(wtile[0:1, 0, 0:1],
                                              x0[0:1, 0, 0:1])
                    for wtile in (w2_sb[0], w2_sb[1]):
                        nc.vector.tensor_copy(wtile[0:1, 0, 0:1],
                                              x0[0:1, 0, 0:1])
                    nc.sync.dma_start(
                        out=wo_sb[0],
                        in_=woT[0].rearrange("(dt p) e -> p dt e", p=128))
                    nc.sync.dma_start(
                        out=w1_sb[0],
                        in_=w1T[0].rearrange("(dt p) e -> p dt e", p=128))
                    nc.sync.dma_start(
                        out=w2_sb[0],
                        in_=w2T[0].rearrange("(ft p) e -> p ft e", p=128))
                    load_weights(1)

                # ---------------- phase 2: transformer ----------------
                with (
                    tc.tile_pool(name="psA", bufs=2, space="PSUM") as psA,
                    tc.tile_pool(name="psS", bufs=2, space="PSUM") as psS,
                    tc.tile_pool(name="psO", bufs=2, space="PSUM") as psO,
                ):
                    def transpose_to(src_f32, dst_bf16):
                        # src: [128, CT, D] f32 token-major; dst: [128, DT, C] bf16
                        for i in range(CT):
                            for j in range(DT):
                                pst = psS.tile([128, 128], f32, tag="ps_t", name="ps_t")
                                nc.tensor.transpose(
                                    pst, src_f32[:, i, j * 128:(j + 1) * 128], ident32)
                                nc.vector.tensor_copy(
                                    dst_bf16[:, j, i * 128:(i + 1) * 128], pst)

                    x_in = x0
                    for l in range(L):
                        xT = xtp.tile([128, DT, C], bf16, tag="xT", name="xT")
                        transpose_to(x_in, xT)

                        # --- q, k feature-major [e, c] ---
                        qT = acts.tile([128, DT, C], bf16, tag="qT", name="qT")
                        kT = acts.tile([128, DT, C], bf16, tag="kT", name="kT")
                        for et in range(8):
                            ps = psA.tile([128, C], f32, tag="ps_a", name="ps_a")
                            for dt_ in range(DT):
                                nc.tensor.matmul(
                                    ps, lhsT=wqkv_sb[l][:, dt_, et * 128:(et + 1) * 128],
                                    rhs=xT[:, dt_, :],
                                    start=(dt_ == 0), stop=(dt_ == DT - 1))
                            dst = qT[:, et, :] if et < 4 else kT[:, et - 4, :]
                            if has_qkv_b:
                                nc.scalar.activation(
                                    out=dst, in_=ps, func=AF.Identity,
                                    bias=bqkv_sb[l][:, et:et + 1], scale=1.0)
                            else:
                                nc.scalar.copy(out=dst, in_=ps)

                        # --- v token-major [c, e] with per-head ones column ---
                        v_ext = acts.tile([128, CT, H, 72], fp8, tag="v_ext",
                                          name="v_ext")
                        nc.vector.memset(v_ext[:, :, :, HD:HD + 1], 1.0)
                        for ct in range(CT):
                            ps = psA.tile([128, C], f32, tag="ps_a", name="ps_a")
                            for dt_ in range(DT):
                                nc.tensor.matmul(
                                    ps, lhsT=xT[:, dt_, ct * 128:(ct + 1) * 128],
                                    rhs=wqkv_sb[l][:, dt_, 2 * D:3 * D],
                                    start=(dt_ == 0), stop=(dt_ == DT - 1))
                            if has_qkv_b:
                                tv = lnp.tile([128, D], f32, tag="ln_t", name="tv")
                                nc.vector.tensor_tensor(out=tv, in0=ps, in1=vb_sb[l],
                                                        op=AL.add)
                                nc.scalar.copy(out=v_ext[:, ct, :, 0:HD], in_=tv)
                            else:
                                nc.scalar.copy(out=v_ext[:, ct, :, 0:HD], in_=ps)

                        # --- attention, software-pipelined across heads ---
                        # PE order: scores(h) ... scores(h+1), av(h), so the PE
                        # never sits behind ACT's exp in its own queue.
                        oT = acts.tile([128, DT, C], bf16, tag="oT", name="oT")
                        expTs = [None] * H     # live expT tiles per head
                        psOs = [None] * H      # live AV psum per head

                        def emit_scores(h):
                            th, off = h // 2, (h % 2) * 64
                            expT = expp.tile([128, CT, C], fp8, tag="expT",
                                             name="expT")
                            for kt in range(CT):
                                ps = psS.tile([128, C], f32, tag="ps_s", name="ps_s")
                                nc.tensor.matmul(
                                    ps,
                                    lhsT=kT[off:off + 64, th, kt * 128:(kt + 1) * 128],
                                    rhs=qT[off:off + 64, th, :], start=True, stop=True)
                                nc.scalar.activation(out=expT[:, kt, :], in_=ps,
                                                     func=AF.Exp, scale=1.0 / 8.0)
                            expTs[h] = expT

                        def emit_av(h):
                            pso = psO.tile([128, C], f32, tag="ps_o", name="ps_o")
                            for kp in range(CT // 2):
                                nc.tensor.matmul(
                                    pso[0:HD + 1, :],
                                    lhsT=v_ext[:, 2 * kp:2 * kp + 2, h, 0:HD + 1],
                                    rhs=expTs[h][:, 2 * kp:2 * kp + 2, :],
                                    start=(kp == 0), stop=(kp == CT // 2 - 1),
                                    perf_mode=DR)
                            psOs[h] = pso

                        def emit_norm(p):
                            # heads 2p (rows 0:64) and 2p+1 (rows 64:128)
                            h0, h1 = 2 * p, 2 * p + 1
                            th = p
                            # denominator rows to SBUF (custom DVE ops must
                            # not read PSUM), ONE fast reciprocal, bf16 cast,
                            # 1-cyc/row bf16 broadcast matmuls into the two
                            # partition halves, single ACT bounce to SBUF,
                            # then two PSUM-direct DVE mults.
                            den2 = nrm.tile([1, 2, C], f32, tag="den2",
                                            name="den2")
                            nc.vector.tensor_copy(den2[:, 0, :],
                                                  psOs[h0][HD:HD + 1, :])
                            nc.vector.tensor_copy(den2[:, 1, :],
                                                  psOs[h1][HD:HD + 1, :])
                            rec2 = nrm.tile([1, 2, C], f32, tag="rec2",
                                            name="rec2")
                            nc.vector.reciprocal_approx_fast(rec2, den2)
                            rec2b = nrm.tile([1, 2, C], bf16, tag="rec2b",
                                             name="rec2b")
                            nc.vector.tensor_copy(rec2b, rec2)
                            psd = psA.tile([128, C], f32, tag="ps_a", name="ps_d")
                            nc.tensor.matmul(
                                psd[0:64, :], lhsT=ones64b, rhs=rec2b[:, 0, :],
                                start=True, stop=True, skip_group_check=True)
                            nc.tensor.matmul(
                                psd[64:128, :], lhsT=ones64b, rhs=rec2b[:, 1, :],
                                start=True, stop=True, skip_group_check=True)
                            rec_bc = nrm.tile([128, C], f32, tag="rec_bc",
                                              name="rec_bc")
                            nc.scalar.copy(out=rec_bc, in_=psd)
                            nc.vector.tensor_tensor(
                                out=oT[0:64, th, :], in0=psOs[h0][0:HD, :],
                                in1=rec_bc[0:64, :], op=AL.mult)
                            nc.vector.tensor_tensor(
                                out=oT[64:128, th, :], in0=psOs[h1][0:HD, :],
                                in1=rec_bc[64:128, :], op=AL.mult)
                            psOs[h0] = psOs[h1] = None

                        emit_scores(0)
                        for h in range(1, H):
                            emit_scores(h)
                            emit_av(h - 1)
                            if h >= 2 and h % 2 == 0:
                                emit_norm((h - 2) // 2)
                        emit_av(H - 1)
                        emit_norm(H // 2 - 1)

                        # --- out-projection + residual + LN1 ---
                        xm2 = xmp.tile([128, CT, D], f32, tag="xm", name="xm2")
                        for ct in range(CT):
                            ps = psA.tile([128, C], f32, tag="ps_a", name="ps_a")
                            for et in range(DT):
                                nc.tensor.matmul(
                                    ps, lhsT=oT[:, et, ct * 128:(ct + 1) * 128],
                                    rhs=wo_sb[l][:, et, :],
                                    start=(et == 0), stop=(et == DT - 1))
                            if has_out_b:
                                nc.vector.tensor_tensor(out=ps, in0=ps, in1=outb_sb[l],
                                                        op=AL.add)
                            ln_block(ps, x_in[:, ct, :],
                                     ln1w_sb[l] if has_ln1 else None,
                                     ln1b_sb[l] if has_ln1 else None,
                                     xm2[:, ct, :])

                        # --- FFN ---
                        x2T = xtp.tile([128, DT, C], bf16, tag="xT", name="x2T")
                        transpose_to(xm2, x2T)
                        hT = acts.tile([128, FT, C], bf16, tag="hT", name="hT")
                        for ft in range(FT):
                            ps = psA.tile([128, C], f32, tag="ps_a", name="ps_a")
                            for dt_ in range(DT):
                                nc.tensor.matmul(
                                    ps, lhsT=w1_sb[l][:, dt_, ft * 128:(ft + 1) * 128],
                                    rhs=x2T[:, dt_, :],
                                    start=(dt_ == 0), stop=(dt_ == DT - 1))
                            nc.scalar.activation(
                                out=hT[:, ft, :], in_=ps, func=AF.Relu,
                                bias=(b1_sb[l][:, ft:ft + 1] if has_ff1_b else 0.0),
                                scale=1.0)
                        x_next = xmp.tile([128, CT, D], f32, tag="xm", name="x_next")
                        for ct in range(CT):
                            ps = psA.tile([128, C], f32, tag="ps_a", name="ps_a")
                            for ft in range(FT):
                                nc.tensor.matmul(
                                    ps, lhsT=hT[:, ft, ct * 128:(ct + 1) * 128],
                                    rhs=w2_sb[l][:, ft, :],
                                    start=(ft == 0), stop=(ft == FT - 1))
                            if has_ff2_b:
                                nc.vector.tensor_tensor(out=ps, in0=ps, in1=ff2b_sb[l],
                                                        op=AL.add)
                            ln_block(ps, xm2[:, ct, :],
                                     ln2w_sb[l] if has_ln2 else None,
                                     ln2b_sb[l] if has_ln2 else None,
                                     x_next[:, ct, :])
                        x_in = x_next

                    # ---------------- phase 3: final LN -> y_bf ----------------
                    for ct in range(CT):
                        ln_block(x_in[:, ct, :], None, flnw_sb, flnb_sb,
                                 y_bf[:, ct, :])

            # ============ scope: expand ============
            # out[s, :] = y[seg[s], :] via one-hot^T matmuls; bf16 output in
            # 1 MiB group DMAs.
            with (
                tc.tile_pool(name="ohp", bufs=3) as ohp,
                tc.tile_pool(name="outp", bufs=2) as outp,
                tc.tile_pool(name="psE", bufs=4, space="PSUM") as psE,
            ):
                seg_row_ap = seg_row[:, :]
                for g in range(NG):
                    seg_bc = ohp.tile([128, GT * 128], f16, tag="seg_bc",
                                      name="seg_bc")
                    src = bass.AP(tensor=seg_row_ap.tensor, offset=g * GT * 128,
                                  ap=[[0, 128], [1, GT * 128]])
                    nc.gpsimd.dma_start(out=seg_bc, in_=src)
                    og = outp.tile([128, GT, D], bf16, tag="og", name="og")
                    for j in range(GT):
                        t = g * GT + j
                        lo, hi = ranges[t]
                        ohT = ohp.tile([128, CT, 128], bf16, tag="ohT", name="ohT")
                        for m in range(lo, hi + 1):
                            nc.vector.tensor_scalar(
                                out=ohT[:, m, :],
                                in0=seg_bc[:, j * 128:(j + 1) * 128],
                                scalar1=iota_col_sb[:, m:m + 1], scalar2=None,
                                op0=AL.is_equal)
                        pse = psE.tile([128, D], f32, tag="ps_e", name="ps_e")
                        for m in range(lo, hi + 1):
                            nc.tensor.matmul(
                                pse, lhsT=ohT[:, m, :],
                                rhs=y_bf[:, m, :],
                                start=(m == lo), stop=(m == hi))
                        if t % 2 == 0:
                            nc.scalar.copy(out=og[:, j, :], in_=pse)
                        else:
                            nc.vector.tensor_copy(og[:, j, :], pse)
                    nc.sync.dma_start(
                        out=out_d[g * GT * 128:(g + 1) * GT * 128, :]
                        .rearrange("(n p) d -> p n d", p=128),
                        in_=og)

    return nc


def _host_prep(inputs):
    """Shard + preprocess full inputs into 8 per-core input maps."""
    bf = ml_dtypes.bfloat16
    f8 = ml_dtypes.float8_e4m3fn
    tokens = np.asarray(inputs["tokens"], dtype=np.float32)
    seg = np.asarray(inputs["segment_ids"], dtype=np.int32)
    qkv_w = np.asarray(inputs["qkv_w"], dtype=np.float32)
    qkv_b = np.asarray(inputs["qkv_b"], dtype=np.float32)
    out_w = np.asarray(inputs["out_w"], dtype=np.float32)
    out_b = np.asarray(inputs["out_b"], dtype=np.float32)
    ln1_w = np.asarray(inputs["ln1_w"], dtype=np.float32)
    ln1_b = np.asarray(inputs["ln1_b"], dtype=np.float32)
    ln2_w = np.asarray(inputs["ln2_w"], dtype=np.float32)
    ln2_b = np.asarray(inputs["ln2_b"], dtype=np.float32)
    ff1_w = np.asarray(inputs["ff1_w"], dtype=np.float32)
    ff1_b = np.asarray(inputs["ff1_b"], dtype=np.float32)
    ff2_w = np.asarray(inputs["ff2_w"], dtype=np.float32)
    ff2_b = np.asarray(inputs["ff2_b"], dtype=np.float32)
    fln_w = np.asarray(inputs["fln_w"], dtype=np.float32)
    fln_b = np.asarray(inputs["fln_b"], dtype=np.float32)

    flags = (
        bool(np.any(qkv_b)),
        bool(np.any(out_b)),
        bool(np.any(ff1_b)),
        bool(np.any(ff2_b)),
        bool(np.any(ln1_w != 1.0) or np.any(ln1_b)),
        bool(np.any(ln2_w != 1.0) or np.any(ln2_b)),
        bool(np.any(fln_w != 1.0) or np.any(fln_b)),
    )

    # span-bound ranges: per token tile, union over batch of the contiguous
    # chunk-tile range its (sorted) segment ids cover.
    srt = np.all(np.diff(seg, axis=1) >= 0)
    if srt:
        lo = np.min(seg[:, ::128] // 128, axis=0)
        hi = np.max(seg[:, 127::128] // 128, axis=0)
    else:  # fallback: no structure assumed
        lo = np.zeros(NT, np.int64)
        hi = np.full(NT, CT - 1, np.int64)
    covered = set()
    for t in range(NT):
        covered.update(range(int(lo[t]), int(hi[t]) + 1))
    if covered != set(range(CT)):
        lo = np.zeros(NT, np.int64)
        hi = np.full(NT, CT - 1, np.int64)
    ranges = tuple((int(lo[t]), int(hi[t])) for t in range(NT))

    # shared (batch-independent) arrays
    shared = {
        "iota_row": np.broadcast_to(
            np.arange(C, dtype=np.float16)[None, :], (128, C)).copy(),
        "iota_col": (np.arange(CT, dtype=np.float32)[None, :] * 128
                     + np.arange(128, dtype=np.float32)[:, None]).astype(np.float32),
        "wqkvT": np.ascontiguousarray(qkv_w.transpose(0, 2, 1)).astype(bf),
        "woT": np.ascontiguousarray(out_w.transpose(0, 2, 1)).astype(bf),
        "w1T": np.ascontiguousarray(ff1_w.transpose(0, 2, 1)).astype(bf),
        "w2T": np.ascontiguousarray(ff2_w.transpose(0, 2, 1)).astype(bf),
    }
    (has_qkv_b, has_out_b, has_ff1_b, has_ff2_b,
     has_ln1, has_ln2, has_fln) = flags
    if has_qkv_b:
        shared["bqkv_c"] = np.ascontiguousarray(
            qkv_b[:, :1536].reshape(L, 12, 128).transpose(0, 2, 1))
        shared["vb_row"] = np.ascontiguousarray(qkv_b[:, 2 * D:3 * D][:, None, :])
    if has_ff1_b:
        shared["b1_c"] = np.ascontiguousarray(
            ff1_b.reshape(L, FT, 128).transpose(0, 2, 1))
    if has_out_b:
        shared["outb_row"] = np.ascontiguousarray(out_b[:, None, :])
    if has_ff2_b:
        shared["ff2b_row"] = np.ascontiguousarray(ff2_b[:, None, :])
    if has_ln1:
        shared["ln1w_row"] = np.ascontiguousarray(ln1_w[:, None, :])
        shared["ln1b_row"] = np.ascontiguousarray(ln1_b[:, None, :])
    if has_ln2:
        shared["ln2w_row"] = np.ascontiguousarray(ln2_w[:, None, :])
        shared["ln2b_row"] = np.ascontiguousarray(ln2_b[:, None, :])
    if has_fln:
        shared["flnw_row"] = np.ascontiguousarray(fln_w[None, :])
        shared["flnb_row"] = np.ascontiguousarray(fln_b[None, :])

    in_maps = []
    for b in range(B):
        m = dict(shared)
        m["tokens_bf"] = np.ascontiguousarray(tokens[b]).astype(bf)
        m["seg_col"] = np.ascontiguousarray(
            seg[b].reshape(NT, 128).T.astype(np.float32))
        m["seg_row"] = np.ascontiguousarray(seg[b].astype(np.float16)[None, :])
        counts = np.bincount(seg[b], minlength=C).astype(np.float32)
        m["inv_cnt"] = np.ascontiguousarray(
            (1.0 / np.maximum(counts, 1.0)).reshape(CT, 128).T)
        in_maps.append(m)
    return flags, ranges, in_maps


def kernel(**inputs) -> np.ndarray:
    from concourse.bass_utils import run_bass_kernel_spmd

    flags, ranges, in_maps = _host_prep(inputs)
    key = (flags, ranges)
    if key not in _CACHE:
        nc = _build(flags, ranges)
        if not nc.is_finalized():
            nc.finalize()
        _CACHE[key] = nc
    nc = _CACHE[key]
    res = run_bass_kernel_spmd(nc, in_maps, list(range(B)))
    return np.stack(
        [np.asarray(res.results[i]["out_bf"]).astype(np.float32)
         for i in range(B)], axis=0)


# revision 35
# speedup vs baseline: 1.0580x; 1.0146x over previous
"""Trainium2 Bass kernel for nn_ChunkProcessor (segment-mean -> 2-layer
transformer encoder over chunks -> gather-expand -> final LN).

Sharding: data-parallel over batch B=8 across the 8 NeuronCores; each core
processes one batch item end to end (no cross-core communication).

v2 design notes (perf):
  - tokens / weights pre-cast to bf16 on HOST; output written bf16 and
    upcast on host: halves all big HBM traffic.
  - 1/counts computed on host -> no count matmuls, no phase-1 reciprocals,
    PSUM banks freed.
  - tokens + output move in 1 MiB group DMAs (8 tiles each) for full DMA BW.
  - attention softmax normalization: denominators from the v ones-column,
    reciprocal_approx_fast on a [2,C] pair tile, broadcast to 128 partitions
    with ONE fp32r matmul per head pair (1 cyc/row), numerators bounced to
    SBUF on ACT, one DVE mult per head. Kills the [1,C] exact reciprocals
    (1.4us each) and fp32 1x64 broadcast matmuls (1us each) of v1.
  - attention software-pipelined: scores for head h+1 are emitted on PE
    before AV of head h so the PE never waits on ACT's exp -- keeps the PE
    HAM clock gate at 2.4 GHz (idle PE re-throttles to 1.2 GHz).
"""

import numpy as np
import ml_dtypes

B, S, D = 8, 8192, 512
C, H, L, DFF = 512, 8, 2, 2048
HD = D // H          # 64
NT = S // 128        # 64 token tiles
CT = C // 128        # 4 chunk tiles
DT = D // 128        # 4 feature tiles
FT = DFF // 128      # 16
GT = 8               # token tiles per DMA group
NG = NT // GT        # token-tile DMA groups
SW = 64.0            # fp8 weight pre-scale (host); folded back as 1/SW
EPS = 1e-5

_CACHE = {}


def _build(flags, ranges):
    """Build the Bass program.

    flags  = (qkv_b, out_b, ff1_b, ff2_b, ln1_aff, ln2_aff, fln_aff) bools.
    ranges = tuple of (lo_m, hi_m) per token tile t: the contiguous range of
             chunk tiles any batch item's tile-t segment ids fall into.
    """
    import concourse.bass as bass
    import concourse.tile as tile
    from concourse import bacc, mybir
    from concourse.masks import make_identity

    (has_qkv_b, has_out_b, has_ff1_b, has_ff2_b,
     has_ln1, has_ln2, has_fln) = flags

    # first/last contributing token tile per chunk tile (for PSUM start/stop)
    first_t = [min(t for t in range(NT) if ranges[t][0] <= m <= ranges[t][1])
               for m in range(CT)]
    last_t = [max(t for t in range(NT) if ranges[t][0] <= m <= ranges[t][1])
              for m in range(CT)]

    f32 = mybir.dt.float32
    f32r = mybir.dt.float32r
    bf16 = mybir.dt.bfloat16
    f16 = mybir.dt.float16
    fp8 = mybir.dt.float8e4
    AL = mybir.AluOpType
    AF = mybir.ActivationFunctionType
    DR = mybir.MatmulPerfMode.DoubleRow

    nc = bacc.Bacc("TRN2", target_bir_lowering=False)

    tokens = nc.declare_dram_parameter("tokens_bf", [S, D], bf16, isOutput=False)
    seg_col = nc.declare_dram_parameter("seg_col", [128, NT], f32, isOutput=False)
    seg_row = nc.declare_dram_parameter("seg_row", [1, S], f16, isOutput=False)
    iota_row = nc.declare_dram_parameter("iota_row", [128, C], f16, isOutput=False)
    iota_col = nc.declare_dram_parameter("iota_col", [128, CT], f32, isOutput=False)
    inv_cnt = nc.declare_dram_parameter("inv_cnt", [128, CT], f32, isOutput=False)
    wqkvT = nc.declare_dram_parameter("wqkvT", [L, D, 3 * D], bf16, isOutput=False)
    woT = nc.declare_dram_parameter("woT", [L, D, D], bf16, isOutput=False)
    w1T = nc.declare_dram_parameter("w1T", [L, D, DFF], bf16, isOutput=False)
    w2T = nc.declare_dram_parameter("w2T", [L, DFF, D], bf16, isOutput=False)
    if has_qkv_b:
        bqkv_c = nc.declare_dram_parameter("bqkv_c", [L, 128, 12], f32, isOutput=False)
        vb_row = nc.declare_dram_parameter("vb_row", [L, 1, D], f32, isOutput=False)
    if has_ff1_b:
        b1_c = nc.declare_dram_parameter("b1_c", [L, 128, FT], f32, isOutput=False)
    if has_out_b:
        outb_row = nc.declare_dram_parameter("outb_row", [L, 1, D], f32, isOutput=False)
    if has_ff2_b:
        ff2b_row = nc.declare_dram_parameter("ff2b_row", [L, 1, D], f32, isOutput=False)
    if has_ln1:
        ln1w_row = nc.declare_dram_parameter("ln1w_row", [L, 1, D], f32, isOutput=False)
        ln1b_row = nc.declare_dram_parameter("ln1b_row", [L, 1, D], f32, isOutput=False)
    if has_ln2:
        ln2w_row = nc.declare_dram_parameter("ln2w_row", [L, 1, D], f32, isOutput=False)
        ln2b_row = nc.declare_dram_parameter("ln2b_row", [L, 1, D], f32, isOutput=False)
    if has_fln:
        flnw_row = nc.declare_dram_parameter("flnw_row", [1, D], f32, isOutput=False)
        flnb_row = nc.declare_dram_parameter("flnb_row", [1, D], f32, isOutput=False)
    out_d = nc.declare_dram_parameter("out_bf", [S, D], bf16, isOutput=True)

    def bcast_load(pool, dram_row, tag):
        """DMA a [1, D] DRAM row into a [128, D] SBUF tile (partition bcast)."""
        t = pool.tile([128, D], f32, tag=tag, name=f"row_{tag}")
        src = bass.AP(tensor=dram_row.tensor, offset=dram_row.offset,
                      ap=[[0, 128]] + [list(p) for p in dram_row.ap[1:]])
        nc.gpsimd.dma_start(out=t, in_=src)
        return t

    with tile.TileContext(nc) as tc:
        with (
            tc.tile_pool(name="consts", bufs=1) as consts,
            tc.tile_pool(name="acts", bufs=1) as acts,
            tc.tile_pool(name="xm", bufs=2) as xmp,
            tc.tile_pool(name="xt", bufs=2) as xtp,
            tc.tile_pool(name="lnp", bufs=2) as lnp,
            tc.tile_pool(name="nrm", bufs=1) as nrm,
            tc.tile_pool(name="rows", bufs=1) as rows,
        ):
            # ---------------- constants ----------------
            seg_col_sb = consts.tile([128, NT], f32)
            nc.gpsimd.dma_start(out=seg_col_sb, in_=seg_col[:, :])
            iota_row_sb = consts.tile([128, C], f16)
            nc.gpsimd.dma_start(out=iota_row_sb, in_=iota_row[:, :])
            iota_col_sb = consts.tile([128, CT], f32)
            nc.gpsimd.dma_start(out=iota_col_sb, in_=iota_col[:, :])
            inv_cnt_sb = consts.tile([128, CT], f32)
            nc.gpsimd.dma_start(out=inv_cnt_sb, in_=inv_cnt[:, :])
            ones64b = consts.tile([1, 64], bf16)
            nc.vector.memset(ones64b, 1.0)
            ones_row32 = consts.tile([1, 128], f32)
            nc.vector.memset(ones_row32, 1.0)
            ident32 = consts.tile([128, 128], f32)
            make_identity(nc, ident32)
            eps_t = consts.tile([128, 1], f32)
            nc.vector.memset(eps_t, EPS)

            # y_bf lives in the persistent pool (used by the expand phase
            # after the weight pool is closed).
            y_bf = acts.tile([128, CT, D], bf16, tag="y_bf")

            def ln_block(ps_src, resid, wrow, brow, out_ap, pre_scale=None):
                # out = LN(ps_src * pre_scale + resid) [* w + b]  (token-major)
                t_ = lnp.tile([128, D], f32, tag="ln_t", name="ln_t")
                if resid is not None and pre_scale is not None:
                    nc.vector.scalar_tensor_tensor(
                        out=t_, in0=ps_src, scalar=pre_scale, in1=resid,
                        op0=AL.mult, op1=AL.add)
                elif resid is not None:
                    nc.vector.tensor_tensor(out=t_, in0=ps_src, in1=resid, op=AL.add)
                else:
                    nc.vector.tensor_copy(t_, ps_src)
                st = lnp.tile([128, 6], f32, tag="ln_st", name="ln_st")
                nc.vector.bn_stats(out=st, in_=t_)
                mv = lnp.tile([128, 2], f32, tag="ln_mv", name="ln_mv")
                nc.vector.bn_aggr(out=mv, in_=st)
                sd = lnp.tile([128, 1], f32, tag="ln_sd", name="ln_sd")
                nc.scalar.activation(out=sd, in_=mv[:, 1:2], func=AF.Sqrt,
                                     bias=eps_t[:, 0:1], scale=1.0)
                rs = lnp.tile([128, 1], f32, tag="ln_rs", name="ln_rs")
                nc.vector.reciprocal(rs, sd)
                if wrow is None:
                    nc.vector.tensor_scalar(
                        out=out_ap, in0=t_, scalar1=mv[:, 0:1], scalar2=rs[:, 0:1],
                        op0=AL.subtract, op1=AL.mult)
                else:
                    xn = lnp.tile([128, D], f32, tag="ln_xn", name="ln_xn")
                    nc.vector.tensor_scalar(
                        out=xn, in0=t_, scalar1=mv[:, 0:1], scalar2=rs[:, 0:1],
                        op0=AL.subtract, op1=AL.mult)
                    nc.vector.tensor_tensor(out=xn, in0=xn, in1=wrow, op=AL.mult)
                    nc.vector.tensor_tensor(out=out_ap, in0=xn, in1=brow, op=AL.add)

            # ============ scope: weights + segsum + transformer ============
            with (
                tc.tile_pool(name="wts", bufs=1) as wts,
                tc.tile_pool(name="expp", bufs=2) as expp,
            ):
                # ---- weights (bf16 in DRAM already) ----
                # All big DMAs share the ONE sync HWDGE ring so ring order =
                # transfer order: an up-front 12MB weight prefetch on its own
                # queue starves the phase-1 token stream (SDMA round-robins
                # between queues; measured: first segsum matmul at 35us).
                # Weight loads are emitted mid-phase-1 instead (see below).
                wqkv_sb = [wts.tile([128, DT, 3 * D], bf16, tag=f"wqkv{l}",
                                    name=f"wqkv{l}") for l in range(L)]
                wo_sb = [wts.tile([128, DT, D], bf16, tag=f"wo{l}",
                                  name=f"wo{l}") for l in range(L)]
                w1_sb = [wts.tile([128, DT, DFF], bf16, tag=f"w1{l}",
                                  name=f"w1{l}") for l in range(L)]
                w2_sb = [wts.tile([128, FT, D], bf16, tag=f"w2{l}",
                                  name=f"w2{l}") for l in range(L)]

                def load_weights(l):
                    nc.sync.dma_start(
                        out=wqkv_sb[l],
                        in_=wqkvT[l].rearrange("(dt p) e -> p dt e", p=128))
                    nc.sync.dma_start(
                        out=wo_sb[l],
                        in_=woT[l].rearrange("(dt p) e -> p dt e", p=128))
                    nc.sync.dma_start(
                        out=w1_sb[l],
                        in_=w1T[l].rearrange("(dt p) e -> p dt e", p=128))
                    nc.sync.dma_start(
                        out=w2_sb[l],
                        in_=w2T[l].rearrange("(ft p) e -> p ft e", p=128))

                bqkv_sb, b1_sb = [], []
                vb_sb, outb_sb, ff2b_sb = [], [], []
                ln1w_sb, ln1b_sb, ln2w_sb, ln2b_sb = [], [], [], []
                for l in range(L):
                    if has_qkv_b:
                        bq = consts.tile([128, 12], f32, tag=f"bqkv{l}", name=f"bqkv{l}")
                        nc.sync.dma_start(out=bq, in_=bqkv_c[l])
                        bqkv_sb.append(bq)
                        vb_sb.append(bcast_load(rows, vb_row[l], f"vb{l}"))
                    if has_ff1_b:
                        b1 = consts.tile([128, FT], f32, tag=f"b1{l}", name=f"b1{l}")
                        nc.sync.dma_start(out=b1, in_=b1_c[l])
                        b1_sb.append(b1)
                    if has_out_b:
                        outb_sb.append(bcast_load(rows, outb_row[l], f"outb{l}"))
                    if has_ff2_b:
                        ff2b_sb.append(bcast_load(rows, ff2b_row[l], f"ff2b{l}"))
                    if has_ln1:
                        ln1w_sb.append(bcast_load(rows, ln1w_row[l], f"ln1w{l}"))
                        ln1b_sb.append(bcast_load(rows, ln1b_row[l], f"ln1b{l}"))
                    if has_ln2:
                        ln2w_sb.append(bcast_load(rows, ln2w_row[l], f"ln2w{l}"))
                        ln2b_sb.append(bcast_load(rows, ln2b_row[l], f"ln2b{l}"))
                flnw_sb = bcast_load(rows, flnw_row, "flnw") if has_fln else None
                flnb_sb = bcast_load(rows, flnb_row, "flnb") if has_fln else None

                # ------------ phase 1: segment sums -> means ------------
                # bf16 token tiles stream over HWDGE in 1 MiB groups of 8;
                # one-hot matmuls accumulate sums in PSUM; host-computed
                # 1/counts turns them into means (no count matmuls).
                x0 = xmp.tile([128, CT, D], f32, tag="xm", name="x0")
                with (
                    tc.tile_pool(name="pseg", bufs=1, space="PSUM") as pseg,
                    tc.tile_pool(name="segs", bufs=2) as segs,
                    tc.tile_pool(name="ohp1", bufs=3) as ohp1,
                ):
                    ps_sums = [pseg.tile([128, D], f32, tag=f"sums{m}", name=f"sums{m}")
                               for m in range(CT)]
                    for g in range(NG):
                        tokg = segs.tile([128, GT, D], bf16, tag="tokg", name="tokg")
                        nc.sync.dma_start(
                            out=tokg,
                            in_=tokens[g * GT * 128:(g + 1) * GT * 128, :]
                            .rearrange("(n p) d -> p n d", p=128))
                        for j in range(GT):
                            t = g * GT + j
                            lo, hi = ranges[t]
                            oh = ohp1.tile([128, C], bf16, tag="oh", name="oh")
                            sl = slice(lo * 128, (hi + 1) * 128)
                            nc.vector.tensor_scalar(
                                out=oh[:, sl], in0=iota_row_sb[:, sl],
                                scalar1=seg_col_sb[:, t:t + 1],
                                scalar2=None, op0=AL.is_equal)
                            for m in range(lo, hi + 1):
                                nc.tensor.matmul(
                                    ps_sums[m], lhsT=oh[:, m * 128:(m + 1) * 128],
                                    rhs=tokg[:, j, :],
                                    start=(t == first_t[m]), stop=(t == last_t[m]))
                        if g == NG - 3:
                            # Ghost WAW dep: the tiny copy into the weight
                            # tile pins the qkv0 DMA AFTER group-3 tokens in
                            # the scheduler (emission order alone is just a
                            # priority hint and gets hoisted).
                            nc.vector.tensor_copy(wqkv_sb[0][0:1, 0, 0:1],
                                                  tokg[0:1, 0, 0:1])
                            nc.sync.dma_start(
                                out=wqkv_sb[0],
                                in_=wqkvT[0].rearrange("(dt p) e -> p dt e",
                                                       p=128))
                    # x = sums * (1/count)
                    for m in range(CT):
                        nc.vector.tensor_scalar(
                            out=x0[:, m, :], in0=ps_sums[m],
                            scalar1=inv_cnt_sb[:, m:m + 1],
                            scalar2=None, op0=AL.mult)
                    # remaining weights stream during phase-2 compute, in ring
                    # order, ghost-dep'd on x0 so they cannot be hoisted into
                    # the token stream.
                    for wtile in (wo_sb[0], w1_sb[0], wqkv_sb[1], wo_sb[1],
                                  w1_sb[1]):
                        nc.vector.tensor_copy(wtile[0:1, 0, 0:1],
                                              x0[0:1, 0, 0:1])
                    for wtile in (w2_sb[0], w2_sb[1]):
                        nc.vector.tensor_copy(wtile[0:1, 0, 0:1],
                                              x0[0:1, 0, 0:1])
                    nc.sync.dma_start(
                        out=wo_sb[0],
                        in_=woT[0].rearrange("(dt p) e -> p dt e", p=128))
                    nc.sync.dma_start(
                        out=w1_sb[0],
                        in_=w1T[0].rearrange("(dt p) e -> p dt e", p=128))
                    nc.sync.dma_start(
                        out=w2_sb[0],
                        in_=w2T[0].rearrange("(ft p) e -> p ft e", p=128))
                    load_weights(1)

                # ---------------- phase 2: transformer ----------------
                with (
                    tc.tile_pool(name="psA", bufs=2, space="PSUM") as psA,
                    tc.tile_pool(name="psS", bufs=2, space="PSUM") as psS,
                    tc.tile_pool(name="psO", bufs=2, space="PSUM") as psO,
                ):
                    def transpose_to(src_f32, dst_bf16):
                        # src: [128, CT, D] f32 token-major; dst: [128, DT, C] bf16
                        for i in range(CT):
                            for j in range(DT):
                                pst = psS.tile([128, 128], f32, tag="ps_t", name="ps_t")
                                nc.tensor.transpose(
                                    pst, src_f32[:, i, j * 128:(j + 1) * 128], ident32)
                                nc.scalar.copy(
                                    out=dst_bf16[:, j, i * 128:(i + 1) * 128],
                                    in_=pst)

                    x_in = x0
                    for l in range(L):
                        xT = xtp.tile([128, DT, C], bf16, tag="xT", name="xT")
                        transpose_to(x_in, xT)

                        # --- q, k feature-major [e, c] ---
                        qT = acts.tile([128, DT, C], bf16, tag="qT", name="qT")
                        kT = acts.tile([128, DT, C], bf16, tag="kT", name="kT")
                        for et in range(8):
                            ps = psA.tile([128, C], f32, tag="ps_a", name="ps_a")
                            for dt_ in range(DT):
                                nc.tensor.matmul(
                                    ps, lhsT=wqkv_sb[l][:, dt_, et * 128:(et + 1) * 128],
                                    rhs=xT[:, dt_, :],
                                    start=(dt_ == 0), stop=(dt_ == DT - 1))
                            dst = qT[:, et, :] if et < 4 else kT[:, et - 4, :]
                            if has_qkv_b:
                                nc.scalar.activation(
                                    out=dst, in_=ps, func=AF.Identity,
                                    bias=bqkv_sb[l][:, et:et + 1], scale=1.0)
                            else:
                                nc.scalar.copy(out=dst, in_=ps)

                        # --- v token-major [c, e] with per-head ones column ---
                        v_ext = acts.tile([128, CT, H, 72], fp8, tag="v_ext",
                                          name="v_ext")
                        nc.vector.memset(v_ext[:, :, :, HD:HD + 1], 1.0)
                        for ct in range(CT):
                            ps = psA.tile([128, C], f32, tag="ps_a", name="ps_a")
                            for dt_ in range(DT):
                                nc.tensor.matmul(
                                    ps, lhsT=xT[:, dt_, ct * 128:(ct + 1) * 128],
                                    rhs=wqkv_sb[l][:, dt_, 2 * D:3 * D],
                                    start=(dt_ == 0), stop=(dt_ == DT - 1))
                            if has_qkv_b:
                                tv = lnp.tile([128, D], f32, tag="ln_t", name="tv")
                                nc.vector.tensor_tensor(out=tv, in0=ps, in1=vb_sb[l],
                                                        op=AL.add)
                                nc.scalar.copy(out=v_ext[:, ct, :, 0:HD], in_=tv)
                            else:
                                nc.scalar.copy(out=v_ext[:, ct, :, 0:HD], in_=ps)

                        # --- attention, software-pipelined across heads ---
                        # PE order: scores(h) ... scores(h+1), av(h), so the PE
                        # never sits behind ACT's exp in its own queue.
                        oT = acts.tile([128, DT, C], bf16, tag="oT", name="oT")
                        expTs = [None] * H     # live expT tiles per head
                        psOs = [None] * H      # live AV psum per head

                        def emit_scores(h):
                            th, off = h // 2, (h % 2) * 64
                            expT = expp.tile([128, CT, C], fp8, tag="expT",
                                             name="expT")
                            for kt in range(CT):
                                ps = psS.tile([128, C], f32, tag="ps_s", name="ps_s")
                                nc.tensor.matmul(
                                    ps,
                                    lhsT=kT[off:off + 64, th, kt * 128:(kt + 1) * 128],
                                    rhs=qT[off:off + 64, th, :], start=True, stop=True)
                                nc.scalar.activation(out=expT[:, kt, :], in_=ps,
                                                     func=AF.Exp, scale=1.0 / 8.0)
                            expTs[h] = expT

                        def emit_av(h):
                            pso = psO.tile([128, C], f32, tag="ps_o", name="ps_o")
                            for kp in range(CT // 2):
                                nc.tensor.matmul(
                                    pso[0:HD + 1, :],
                                    lhsT=v_ext[:, 2 * kp:2 * kp + 2, h, 0:HD + 1],
                                    rhs=expTs[h][:, 2 * kp:2 * kp + 2, :],
                                    start=(kp == 0), stop=(kp == CT // 2 - 1),
                                    perf_mode=DR)
                            psOs[h] = pso

                        def emit_norm(p):
                            # heads 2p (rows 0:64) and 2p+1 (rows 64:128)
                            h0, h1 = 2 * p, 2 * p + 1
                            th = p
                            # denominator rows to SBUF (custom DVE ops must
                            # not read PSUM), ONE fast reciprocal, bf16 cast,
                            # 1-cyc/row bf16 broadcast matmuls into the two
                            # partition halves, single ACT bounce to SBUF,
                            # then two PSUM-direct DVE mults.
                            den2 = nrm.tile([1, 2, C], f32, tag="den2",
                                            name="den2")
                            nc.vector.tensor_copy(den2[:, 0, :],
                                                  psOs[h0][HD:HD + 1, :])
                            nc.vector.tensor_copy(den2[:, 1, :],
                                                  psOs[h1][HD:HD + 1, :])
                            rec2 = nrm.tile([1, 2, C], f32, tag="rec2",
                                            name="rec2")
                            nc.vector.reciprocal_approx_fast(rec2, den2)
                            rec2b = nrm.tile([1, 2, C], bf16, tag="rec2b",
                                             name="rec2b")
                            nc.vector.tensor_copy(rec2b, rec2)
                            psd = psA.tile([128, C], f32, tag="ps_a", name="ps_d")
                            nc.tensor.matmul(
                                psd[0:64, :], lhsT=ones64b, rhs=rec2b[:, 0, :],
                                start=True, stop=True, skip_group_check=True)
                            nc.tensor.matmul(
                                psd[64:128, :], lhsT=ones64b, rhs=rec2b[:, 1, :],
                                start=True, stop=True, skip_group_check=True)
                            rec_bc = nrm.tile([128, C], f32, tag="rec_bc",
                                              name="rec_bc")
                            nc.scalar.copy(out=rec_bc, in_=psd)
                            nc.vector.tensor_tensor(
                                out=oT[0:64, th, :], in0=psOs[h0][0:HD, :],
                                in1=rec_bc[0:64, :], op=AL.mult)
                            nc.vector.tensor_tensor(
                                out=oT[64:128, th, :], in0=psOs[h1][0:HD, :],
                                in1=rec_bc[64:128, :], op=AL.mult)
                            psOs[h0] = psOs[h1] = None

                        emit_scores(0)
                        for h in range(1, H):
                            emit_scores(h)
                            emit_av(h - 1)
                            if h >= 2 and h % 2 == 0:
                                emit_norm((h - 2) // 2)
                        emit_av(H - 1)
                        emit_norm(H // 2 - 1)

                        # --- out-projection + residual + LN1 ---
                        xm2 = xmp.tile([128, CT, D], f32, tag="xm", name="xm2")
                        for ct in range(CT):
                            ps = psA.tile([128, C], f32, tag="ps_a", name="ps_a")
                            for et in range(DT):
                                nc.tensor.matmul(
                                    ps, lhsT=oT[:, et, ct * 128:(ct + 1) * 128],
                                    rhs=wo_sb[l][:, et, :],
                                    start=(et == 0), stop=(et == DT - 1))
                            if has_out_b:
                                nc.vector.tensor_tensor(out=ps, in0=ps, in1=outb_sb[l],
                                                        op=AL.add)
                            ln_block(ps, x_in[:, ct, :],
                                     ln1w_sb[l] if has_ln1 else None,
                                     ln1b_sb[l] if has_ln1 else None,
                                     xm2[:, ct, :])

                        # --- FFN ---
                        x2T = xtp.tile([128, DT, C], bf16, tag="xT", name="x2T")
                        transpose_to(xm2, x2T)
                        hT = acts.tile([128, FT, C], bf16, tag="hT", name="hT")
                        for ft in range(FT):
                            ps = psA.tile([128, C], f32, tag="ps_a", name="ps_a")
                            for dt_ in range(DT):
                                nc.tensor.matmul(
                                    ps, lhsT=w1_sb[l][:, dt_, ft * 128:(ft + 1) * 128],
                                    rhs=x2T[:, dt_, :],
                                    start=(dt_ == 0), stop=(dt_ == DT - 1))
                            nc.scalar.activation(
                                out=hT[:, ft, :], in_=ps, func=AF.Relu,
                                bias=(b1_sb[l][:, ft:ft + 1] if has_ff1_b else 0.0),
                                scale=1.0)
                        x_next = xmp.tile([128, CT, D], f32, tag="xm", name="x_next")
                        for ct in range(CT):
                            ps = psA.tile([128, C], f32, tag="ps_a", name="ps_a")
                            for ft in range(FT):
                                nc.tensor.matmul(
                                    ps, lhsT=hT[:, ft, ct * 128:(ct + 1) * 128],
                                    rhs=w2_sb[l][:, ft, :],
                                    start=(ft == 0), stop=(ft == FT - 1))
                            if has_ff2_b:
                                nc.vector.tensor_tensor(out=ps, in0=ps, in1=ff2b_sb[l],
                                                        op=AL.add)
                            ln_block(ps, xm2[:, ct, :],
                                     ln2w_sb[l] if has_ln2 else None,
                                     ln2b_sb[l] if has_ln2 else None,
                                     x_next[:, ct, :])
                        x_in = x_next

                    # ---------------- phase 3: final LN -> y_bf ----------------
                    for ct in range(CT):
                        ln_block(x_in[:, ct, :], None, flnw_sb, flnb_sb,
                                 y_bf[:, ct, :])

            # ============ scope: expand ============
            # out[s, :] = y[seg[s], :] via one-hot^T matmuls; bf16 output in
            # 1 MiB group DMAs.
            with (
                tc.tile_pool(name="ohp", bufs=4) as ohp,
                tc.tile_pool(name="outp", bufs=3) as outp,
                tc.tile_pool(name="psE", bufs=4, space="PSUM") as psE,
            ):
                seg_row_ap = seg_row[:, :]
                for g in range(NG):
                    seg_bc = ohp.tile([128, GT * 128], f16, tag="seg_bc",
                                      name="seg_bc")
                    src = bass.AP(tensor=seg_row_ap.tensor, offset=g * GT * 128,
                                  ap=[[0, 128], [1, GT * 128]])
                    nc.gpsimd.dma_start(out=seg_bc, in_=src)
                    og = outp.tile([128, GT, D], bf16, tag="og", name="og")
                    for j in range(GT):
                        t = g * GT + j
                        lo, hi = ranges[t]
                        ohT = ohp.tile([128, CT, 128], bf16, tag="ohT", name="ohT")
                        for m in range(lo, hi + 1):
                            nc.vector.tensor_scalar(
                                out=ohT[:, m, :],
                                in0=seg_bc[:, j * 128:(j + 1) * 128],
                                scalar1=iota_col_sb[:, m:m + 1], scalar2=None,
                                op0=AL.is_equal)
                        pse = psE.tile([128, D], f32, tag="ps_e", name="ps_e")
                        for m in range(lo, hi + 1):
                            nc.tensor.matmul(
                                pse, lhsT=ohT[:, m, :],
                                rhs=y_bf[:, m, :],
                                start=(m == lo), stop=(m == hi))
                        if t % 2 == 0:
                            nc.scalar.copy(out=og[:, j, :], in_=pse)
                        else:
                            nc.vector.tensor_copy(og[:, j, :], pse)
                    nc.sync.dma_start(
                        out=out_d[g * GT * 128:(g + 1) * GT * 128, :]
                        .rearrange("(n p) d -> p n d", p=128),
                        in_=og)

    return nc


def _host_prep(inputs):
    """Shard + preprocess full inputs into 8 per-core input maps."""
    bf = ml_dtypes.bfloat16
    f8 = ml_dtypes.float8_e4m3fn
    tokens = np.asarray(inputs["tokens"], dtype=np.float32)
    seg = np.asarray(inputs["segment_ids"], dtype=np.int32)
    qkv_w = np.asarray(inputs["qkv_w"], dtype=np.float32)
    qkv_b = np.asarray(inputs["qkv_b"], dtype=np.float32)
    out_w = np.asarray(inputs["out_w"], dtype=np.float32)
    out_b = np.asarray(inputs["out_b"], dtype=np.float32)
    ln1_w = np.asarray(inputs["ln1_w"], dtype=np.float32)
    ln1_b = np.asarray(inputs["ln1_b"], dtype=np.float32)
    ln2_w = np.asarray(inputs["ln2_w"], dtype=np.float32)
    ln2_b = np.asarray(inputs["ln2_b"], dtype=np.float32)
    ff1_w = np.asarray(inputs["ff1_w"], dtype=np.float32)
    ff1_b = np.asarray(inputs["ff1_b"], dtype=np.float32)
    ff2_w = np.asarray(inputs["ff2_w"], dtype=np.float32)
    ff2_b = np.asarray(inputs["ff2_b"], dtype=np.float32)
    fln_w = np.asarray(inputs["fln_w"], dtype=np.float32)
    fln_b = np.asarray(inputs["fln_b"], dtype=np.float32)

    flags = (
        bool(np.any(qkv_b)),
        bool(np.any(out_b)),
        bool(np.any(ff1_b)),
        bool(np.any(ff2_b)),
        bool(np.any(ln1_w != 1.0) or np.any(ln1_b)),
        bool(np.any(ln2_w != 1.0) or np.any(ln2_b)),
        bool(np.any(fln_w != 1.0) or np.any(fln_b)),
    )

    # span-bound ranges: per token tile, union over batch of the contiguous
    # chunk-tile range its (sorted) segment ids cover.
    srt = np.all(np.diff(seg, axis=1) >= 0)
    if srt:
        lo = np.min(seg[:, ::128] // 128, axis=0)
        hi = np.max(seg[:, 127::128] // 128, axis=0)
    else:  # fallback: no structure assumed
        lo = np.zeros(NT, np.int64)
        hi = np.full(NT, CT - 1, np.int64)
    covered = set()
    for t in range(NT):
        covered.update(range(int(lo[t]), int(hi[t]) + 1))
    if covered != set(range(CT)):
        lo = np.zeros(NT, np.int64)
        hi = np.full(NT, CT - 1, np.int64)
    ranges = tuple((int(lo[t]), int(hi[t])) for t in range(NT))

    # shared (batch-independent) arrays
    shared = {
        "iota_row": np.broadcast_to(
            np.arange(C, dtype=np.float16)[None, :], (128, C)).copy(),
        "iota_col": (np.arange(CT, dtype=np.float32)[None, :] * 128
                     + np.arange(128, dtype=np.float32)[:, None]).astype(np.float32),
        "wqkvT": np.ascontiguousarray(qkv_w.transpose(0, 2, 1)).astype(bf),
        "woT": np.ascontiguousarray(out_w.transpose(0, 2, 1)).astype(bf),
        "w1T": np.ascontiguousarray(ff1_w.transpose(0, 2, 1)).astype(bf),
        "w2T": np.ascontiguousarray(ff2_w.transpose(0, 2, 1)).astype(bf),
    }
    (has_qkv_b, has_out_b, has_ff1_b, has_ff2_b,
     has_ln1, has_ln2, has_fln) = flags
    if has_qkv_b:
        shared["bqkv_c"] = np.ascontiguousarray(
            qkv_b[:, :1536].reshape(L, 12, 128).transpose(0, 2, 1))
        shared["vb_row"] = np.ascontiguousarray(qkv_b[:, 2 * D:3 * D][:, None, :])
    if has_ff1_b:
        shared["b1_c"] = np.ascontiguousarray(
            ff1_b.reshape(L, FT, 128).transpose(0, 2, 1))
    if has_out_b:
        shared["outb_row"] = np.ascontiguousarray(out_b[:, None, :])
    if has_ff2_b:
        shared["ff2b_row"] = np.ascontiguousarray(ff2_b[:, None, :])
    if has_ln1:
        shared["ln1w_row"] = np.ascontiguousarray(ln1_w[:, None, :])
        shared["ln1b_row"] = np.ascontiguousarray(ln1_b[:, None, :])
    if has_ln2:
        shared["ln2w_row"] = np.ascontiguousarray(ln2_w[:, None, :])
        shared["ln2b_row"] = np.ascontiguousarray(ln2_b[:, None, :])
    if has_fln:
        shared["flnw_row"] = np.ascontiguousarray(fln_w[None, :])
        shared["flnb_row"] = np.ascontiguousarray(fln_b[None, :])

    in_maps = []
    for b in range(B):
        m = dict(shared)
        m["tokens_bf"] = np.ascontiguousarray(tokens[b]).astype(bf)
        m["seg_col"] = np.ascontiguousarray(
            seg[b].reshape(NT, 128).T.astype(np.float32))
        m["seg_row"] = np.ascontiguousarray(seg[b].astype(np.float16)[None, :])
        counts = np.bincount(seg[b], minlength=C).astype(np.float32)
        m["inv_cnt"] = np.ascontiguousarray(
            (1.0 / np.maximum(counts, 1.0)).reshape(CT, 128).T)
        in_maps.append(m)
    return flags, ranges, in_maps


def kernel(**inputs) -> np.ndarray:
    from concourse.bass_utils import run_bass_kernel_spmd

    flags, ranges, in_maps = _host_prep(inputs)
    key = (flags, ranges)
    if key not in _CACHE:
        nc = _build(flags, ranges)
        if not nc.is_finalized():
            nc.finalize()
        _CACHE[key] = nc
    nc = _CACHE[key]
    res = run_bass_kernel_spmd(nc, in_maps, list(range(B)))
    return np.stack(
        [np.asarray(res.results[i]["out_bf"]).astype(np.float32)
         for i in range(B)], axis=0)


# revision 36
# speedup vs baseline: 1.1301x; 1.0681x over previous
"""Trainium2 Bass kernel for nn_ChunkProcessor (segment-mean -> 2-layer
transformer encoder over chunks -> gather-expand -> final LN).

Sharding: data-parallel over batch B=8 across the 8 NeuronCores; each core
processes one batch item end to end (no cross-core communication).

v2 design notes (perf):
  - tokens / weights pre-cast to bf16 on HOST; output written bf16 and
    upcast on host: halves all big HBM traffic.
  - 1/counts computed on host -> no count matmuls, no phase-1 reciprocals,
    PSUM banks freed.
  - tokens + output move in 1 MiB group DMAs (8 tiles each) for full DMA BW.
  - attention softmax normalization: denominators from the v ones-column,
    reciprocal_approx_fast on a [2,C] pair tile, broadcast to 128 partitions
    with ONE fp32r matmul per head pair (1 cyc/row), numerators bounced to
    SBUF on ACT, one DVE mult per head. Kills the [1,C] exact reciprocals
    (1.4us each) and fp32 1x64 broadcast matmuls (1us each) of v1.
  - attention software-pipelined: scores for head h+1 are emitted on PE
    before AV of head h so the PE never waits on ACT's exp -- keeps the PE
    HAM clock gate at 2.4 GHz (idle PE re-throttles to 1.2 GHz).
"""

import numpy as np
import ml_dtypes

B, S, D = 8, 8192, 512
C, H, L, DFF = 512, 8, 2, 2048
HD = D // H          # 64
NT = S // 128        # 64 token tiles
CT = C // 128        # 4 chunk tiles
DT = D // 128        # 4 feature tiles
FT = DFF // 128      # 16
GT = 8               # token tiles per DMA group
NG = NT // GT        # token-tile DMA groups
SW = 64.0            # fp8 weight pre-scale (host); folded back as 1/SW
EPS = 1e-5

_CACHE = {}


def _build(flags, ranges):
    """Build the Bass program.

    flags  = (qkv_b, out_b, ff1_b, ff2_b, ln1_aff, ln2_aff, fln_aff) bools.
    ranges = tuple of (lo_m, hi_m) per token tile t: the contiguous range of
             chunk tiles any batch item's tile-t segment ids fall into.
    """
    import concourse.bass as bass
    import concourse.tile as tile
    from concourse import bacc, mybir
    from concourse.masks import make_identity

    (has_qkv_b, has_out_b, has_ff1_b, has_ff2_b,
     has_ln1, has_ln2, has_fln) = flags

    # first/last contributing token tile per chunk tile (for PSUM start/stop)
    first_t = [min(t for t in range(NT) if ranges[t][0] <= m <= ranges[t][1])
               for m in range(CT)]
    last_t = [max(t for t in range(NT) if ranges[t][0] <= m <= ranges[t][1])
              for m in range(CT)]

    f32 = mybir.dt.float32
    f32r = mybir.dt.float32r
    bf16 = mybir.dt.bfloat16
    f16 = mybir.dt.float16
    fp8 = mybir.dt.float8e4
    AL = mybir.AluOpType
    AF = mybir.ActivationFunctionType
    DR = mybir.MatmulPerfMode.DoubleRow

    nc = bacc.Bacc("TRN2", target_bir_lowering=False)

    tokens = nc.declare_dram_parameter("tokens_bf", [S, D], bf16, isOutput=False)
    seg_col = nc.declare_dram_parameter("seg_col", [128, NT], f32, isOutput=False)
    seg_row = nc.declare_dram_parameter("seg_row", [1, S], f16, isOutput=False)
    iota_row = nc.declare_dram_parameter("iota_row", [128, C], f16, isOutput=False)
    iota_col = nc.declare_dram_parameter("iota_col", [128, CT], f32, isOutput=False)
    inv_cnt = nc.declare_dram_parameter("inv_cnt", [128, CT], f32, isOutput=False)
    wqkvT = nc.declare_dram_parameter("wqkvT", [L, D, 3 * D], bf16, isOutput=False)
    woT = nc.declare_dram_parameter("woT", [L, D, D], bf16, isOutput=False)
    w1T = nc.declare_dram_parameter("w1T", [L, D, DFF], bf16, isOutput=False)
    w2T = nc.declare_dram_parameter("w2T", [L, DFF, D], bf16, isOutput=False)
    if has_qkv_b:
        bqkv_c = nc.declare_dram_parameter("bqkv_c", [L, 128, 12], f32, isOutput=False)
        vb_row = nc.declare_dram_parameter("vb_row", [L, 1, D], f32, isOutput=False)
    if has_ff1_b:
        b1_c = nc.declare_dram_parameter("b1_c", [L, 128, FT], f32, isOutput=False)
    if has_out_b:
        outb_row = nc.declare_dram_parameter("outb_row", [L, 1, D], f32, isOutput=False)
    if has_ff2_b:
        ff2b_row = nc.declare_dram_parameter("ff2b_row", [L, 1, D], f32, isOutput=False)
    if has_ln1:
        ln1w_row = nc.declare_dram_parameter("ln1w_row", [L, 1, D], f32, isOutput=False)
        ln1b_row = nc.declare_dram_parameter("ln1b_row", [L, 1, D], f32, isOutput=False)
    if has_ln2:
        ln2w_row = nc.declare_dram_parameter("ln2w_row", [L, 1, D], f32, isOutput=False)
        ln2b_row = nc.declare_dram_parameter("ln2b_row", [L, 1, D], f32, isOutput=False)
    if has_fln:
        flnw_row = nc.declare_dram_parameter("flnw_row", [1, D], f32, isOutput=False)
        flnb_row = nc.declare_dram_parameter("flnb_row", [1, D], f32, isOutput=False)
    out_d = nc.declare_dram_parameter("out_bf", [S, D], bf16, isOutput=True)

    def bcast_load(pool, dram_row, tag):
        """DMA a [1, D] DRAM row into a [128, D] SBUF tile (partition bcast)."""
        t = pool.tile([128, D], f32, tag=tag, name=f"row_{tag}")
        src = bass.AP(tensor=dram_row.tensor, offset=dram_row.offset,
                      ap=[[0, 128]] + [list(p) for p in dram_row.ap[1:]])
        nc.gpsimd.dma_start(out=t, in_=src)
        return t

    with tile.TileContext(nc) as tc:
        with (
            tc.tile_pool(name="consts", bufs=1) as consts,
            tc.tile_pool(name="acts", bufs=1) as acts,
            tc.tile_pool(name="xm", bufs=2) as xmp,
            tc.tile_pool(name="xt", bufs=2) as xtp,
            tc.tile_pool(name="lnp", bufs=2) as lnp,
            tc.tile_pool(name="nrm", bufs=1) as nrm,
            tc.tile_pool(name="rows", bufs=1) as rows,
        ):
            # ---------------- constants ----------------
            seg_col_sb = consts.tile([128, NT], f32)
            nc.gpsimd.dma_start(out=seg_col_sb, in_=seg_col[:, :])
            iota_row_sb = consts.tile([128, C], f16)
            nc.gpsimd.dma_start(out=iota_row_sb, in_=iota_row[:, :])
            iota_col_sb = consts.tile([128, CT], f32)
            nc.gpsimd.dma_start(out=iota_col_sb, in_=iota_col[:, :])
            inv_cnt_sb = consts.tile([128, CT], f32)
            nc.gpsimd.dma_start(out=inv_cnt_sb, in_=inv_cnt[:, :])
            ones64b = consts.tile([1, 64], bf16)
            nc.vector.memset(ones64b, 1.0)
            ones_row32 = consts.tile([1, 128], f32)
            nc.vector.memset(ones_row32, 1.0)
            ident32 = consts.tile([128, 128], f32)
            make_identity(nc, ident32)
            eps_t = consts.tile([128, 1], f32)
            nc.vector.memset(eps_t, EPS)

            # y_bf lives in the persistent pool (used by the expand phase
            # after the weight pool is closed).
            y_bf = acts.tile([128, CT, D], bf16, tag="y_bf")
            # group-0 expand one-hot source, prefetched during phase 2 so the
            # expand phase does not start with a cold DMA latency chain.
            seg_bc0 = acts.tile([128, GT * 128], f16, tag="seg_bc0")
            nc.gpsimd.dma_start(
                out=seg_bc0,
                in_=bass.AP(tensor=seg_row[:, :].tensor, offset=0,
                            ap=[[0, 128], [1, GT * 128]]))

            def ln_block(ps_src, resid, wrow, brow, out_ap, pre_scale=None):
                # out = LN(ps_src * pre_scale + resid) [* w + b]  (token-major)
                t_ = lnp.tile([128, D], f32, tag="ln_t", name="ln_t")
                if resid is not None and pre_scale is not None:
                    nc.vector.scalar_tensor_tensor(
                        out=t_, in0=ps_src, scalar=pre_scale, in1=resid,
                        op0=AL.mult, op1=AL.add)
                elif resid is not None:
                    nc.vector.tensor_tensor(out=t_, in0=ps_src, in1=resid, op=AL.add)
                else:
                    nc.vector.tensor_copy(t_, ps_src)
                st = lnp.tile([128, 6], f32, tag="ln_st", name="ln_st")
                nc.vector.bn_stats(out=st, in_=t_)
                mv = lnp.tile([128, 2], f32, tag="ln_mv", name="ln_mv")
                nc.vector.bn_aggr(out=mv, in_=st)
                sd = lnp.tile([128, 1], f32, tag="ln_sd", name="ln_sd")
                nc.scalar.activation(out=sd, in_=mv[:, 1:2], func=AF.Sqrt,
                                     bias=eps_t[:, 0:1], scale=1.0)
                rs = lnp.tile([128, 1], f32, tag="ln_rs", name="ln_rs")
                nc.vector.reciprocal(rs, sd)
                if wrow is None:
                    nc.vector.tensor_scalar(
                        out=out_ap, in0=t_, scalar1=mv[:, 0:1], scalar2=rs[:, 0:1],
                        op0=AL.subtract, op1=AL.mult)
                else:
                    xn = lnp.tile([128, D], f32, tag="ln_xn", name="ln_xn")
                    nc.vector.tensor_scalar(
                        out=xn, in0=t_, scalar1=mv[:, 0:1], scalar2=rs[:, 0:1],
                        op0=AL.subtract, op1=AL.mult)
                    nc.vector.tensor_tensor(out=xn, in0=xn, in1=wrow, op=AL.mult)
                    nc.vector.tensor_tensor(out=out_ap, in0=xn, in1=brow, op=AL.add)

            # ============ scope: weights + segsum + transformer ============
            with (
                tc.tile_pool(name="wts", bufs=1) as wts,
                tc.tile_pool(name="expp", bufs=2) as expp,
            ):
                # ---- weights (bf16 in DRAM already) ----
                # All big DMAs share the ONE sync HWDGE ring so ring order =
                # transfer order: an up-front 12MB weight prefetch on its own
                # queue starves the phase-1 token stream (SDMA round-robins
                # between queues; measured: first segsum matmul at 35us).
                # Weight loads are emitted mid-phase-1 instead (see below).
                wqkv_sb = [wts.tile([128, DT, 3 * D], bf16, tag=f"wqkv{l}",
                                    name=f"wqkv{l}") for l in range(L)]
                wo_sb = [wts.tile([128, DT, D], bf16, tag=f"wo{l}",
                                  name=f"wo{l}") for l in range(L)]
                w1_sb = [wts.tile([128, DT, DFF], bf16, tag=f"w1{l}",
                                  name=f"w1{l}") for l in range(L)]
                w2_sb = [wts.tile([128, FT, D], bf16, tag=f"w2{l}",
                                  name=f"w2{l}") for l in range(L)]

                def load_weights(l):
                    nc.sync.dma_start(
                        out=wqkv_sb[l],
                        in_=wqkvT[l].rearrange("(dt p) e -> p dt e", p=128))
                    nc.sync.dma_start(
                        out=wo_sb[l],
                        in_=woT[l].rearrange("(dt p) e -> p dt e", p=128))
                    nc.sync.dma_start(
                        out=w1_sb[l],
                        in_=w1T[l].rearrange("(dt p) e -> p dt e", p=128))
                    nc.sync.dma_start(
                        out=w2_sb[l],
                        in_=w2T[l].rearrange("(ft p) e -> p ft e", p=128))

                bqkv_sb, b1_sb = [], []
                vb_sb, outb_sb, ff2b_sb = [], [], []
                ln1w_sb, ln1b_sb, ln2w_sb, ln2b_sb = [], [], [], []
                for l in range(L):
                    if has_qkv_b:
                        bq = consts.tile([128, 12], f32, tag=f"bqkv{l}", name=f"bqkv{l}")
                        nc.sync.dma_start(out=bq, in_=bqkv_c[l])
                        bqkv_sb.append(bq)
                        vb_sb.append(bcast_load(rows, vb_row[l], f"vb{l}"))
                    if has_ff1_b:
                        b1 = consts.tile([128, FT], f32, tag=f"b1{l}", name=f"b1{l}")
                        nc.sync.dma_start(out=b1, in_=b1_c[l])
                        b1_sb.append(b1)
                    if has_out_b:
                        outb_sb.append(bcast_load(rows, outb_row[l], f"outb{l}"))
                    if has_ff2_b:
                        ff2b_sb.append(bcast_load(rows, ff2b_row[l], f"ff2b{l}"))
                    if has_ln1:
                        ln1w_sb.append(bcast_load(rows, ln1w_row[l], f"ln1w{l}"))
                        ln1b_sb.append(bcast_load(rows, ln1b_row[l], f"ln1b{l}"))
                    if has_ln2:
                        ln2w_sb.append(bcast_load(rows, ln2w_row[l], f"ln2w{l}"))
                        ln2b_sb.append(bcast_load(rows, ln2b_row[l], f"ln2b{l}"))
                flnw_sb = bcast_load(rows, flnw_row, "flnw") if has_fln else None
                flnb_sb = bcast_load(rows, flnb_row, "flnb") if has_fln else None

                # ------------ phase 1: segment sums -> means ------------
                # bf16 token tiles stream over HWDGE in 1 MiB groups of 8;
                # one-hot matmuls accumulate sums in PSUM; host-computed
                # 1/counts turns them into means (no count matmuls).
                x0 = xmp.tile([128, CT, D], f32, tag="xm", name="x0")
                with (
                    tc.tile_pool(name="pseg", bufs=1, space="PSUM") as pseg,
                    tc.tile_pool(name="segs", bufs=2) as segs,
                    tc.tile_pool(name="ohp1", bufs=3) as ohp1,
                ):
                    ps_sums = [pseg.tile([128, D], f32, tag=f"sums{m}", name=f"sums{m}")
                               for m in range(CT)]
                    for g in range(NG):
                        tokg = segs.tile([128, GT, D], bf16, tag="tokg", name="tokg")
                        nc.sync.dma_start(
                            out=tokg,
                            in_=tokens[g * GT * 128:(g + 1) * GT * 128, :]
                            .rearrange("(n p) d -> p n d", p=128))
                        for j in range(GT):
                            t = g * GT + j
                            lo, hi = ranges[t]
                            oh = ohp1.tile([128, C], bf16, tag="oh", name="oh")
                            sl = slice(lo * 128, (hi + 1) * 128)
                            nc.vector.tensor_scalar(
                                out=oh[:, sl], in0=iota_row_sb[:, sl],
                                scalar1=seg_col_sb[:, t:t + 1],
                                scalar2=None, op0=AL.is_equal)
                            for m in range(lo, hi + 1):
                                nc.tensor.matmul(
                                    ps_sums[m], lhsT=oh[:, m * 128:(m + 1) * 128],
                                    rhs=tokg[:, j, :],
                                    start=(t == first_t[m]), stop=(t == last_t[m]))
                        if g >= NG - 4:
                            # Ghost WAW dep: the tiny copy into the weight
                            # tile pins this qkv0 chunk AFTER group-g tokens
                            # in the scheduler (emission order alone is just
                            # a priority hint and gets hoisted). One 384KB
                            # dt-chunk rides between token groups so tokens
                            # never wait behind a monolithic 1.5MB load.
                            dt_c = g - (NG - 4)
                            nc.vector.tensor_copy(
                                wqkv_sb[0][0:1, dt_c, 0:1], tokg[0:1, 0, 0:1])
                            nc.sync.dma_start(
                                out=wqkv_sb[0][:, dt_c, :],
                                in_=wqkvT[0][dt_c * 128:(dt_c + 1) * 128, :]
                                .rearrange("p e -> p e"))
                    # x = sums * (1/count)
                    for m in range(CT):
                        nc.vector.tensor_scalar(
                            out=x0[:, m, :], in0=ps_sums[m],
                            scalar1=inv_cnt_sb[:, m:m + 1],
                            scalar2=None, op0=AL.mult)
                    # remaining weights stream during phase-2 compute, in ring
                    # order, ghost-dep'd on x0 so they cannot be hoisted into
                    # the token stream.
                    for wtile in (wo_sb[0], w1_sb[0], wqkv_sb[1], wo_sb[1],
                                  w1_sb[1]):
                        nc.vector.tensor_copy(wtile[0:1, 0, 0:1],
                                              x0[0:1, 0, 0:1])
                    for wtile in (w2_sb[0], w2_sb[1]):
                        nc.vector.tensor_copy(wtile[0:1, 0, 0:1],
                                              x0[0:1, 0, 0:1])
                    nc.sync.dma_start(
                        out=wo_sb[0],
                        in_=woT[0].rearrange("(dt p) e -> p dt e", p=128))
                    nc.sync.dma_start(
                        out=w1_sb[0],
                        in_=w1T[0].rearrange("(dt p) e -> p dt e", p=128))
                    nc.sync.dma_start(
                        out=w2_sb[0],
                        in_=w2T[0].rearrange("(ft p) e -> p ft e", p=128))
                    load_weights(1)

                # ---------------- phase 2: transformer ----------------
                with (
                    tc.tile_pool(name="psA", bufs=2, space="PSUM") as psA,
                    tc.tile_pool(name="psS", bufs=2, space="PSUM") as psS,
                    tc.tile_pool(name="psO", bufs=2, space="PSUM") as psO,
                ):
                    def transpose_to(src_f32, dst_bf16):
                        # src: [128, CT, D] f32 token-major; dst: [128, DT, C] bf16
                        for i in range(CT):
                            for j in range(DT):
                                pst = psS.tile([128, 128], f32, tag="ps_t", name="ps_t")
                                nc.tensor.transpose(
                                    pst, src_f32[:, i, j * 128:(j + 1) * 128], ident32)
                                nc.scalar.copy(
                                    out=dst_bf16[:, j, i * 128:(i + 1) * 128],
                                    in_=pst)

                    x_in = x0
                    for l in range(L):
                        xT = xtp.tile([128, DT, C], bf16, tag="xT", name="xT")
                        transpose_to(x_in, xT)

                        # --- q, k feature-major [e, c] ---
                        qT = acts.tile([128, DT, C], bf16, tag="qT", name="qT")
                        kT = acts.tile([128, DT, C], bf16, tag="kT", name="kT")
                        for et in range(8):
                            ps = psA.tile([128, C], f32, tag="ps_a", name="ps_a")
                            for dt_ in range(DT):
                                nc.tensor.matmul(
                                    ps, lhsT=wqkv_sb[l][:, dt_, et * 128:(et + 1) * 128],
                                    rhs=xT[:, dt_, :],
                                    start=(dt_ == 0), stop=(dt_ == DT - 1))
                            dst = qT[:, et, :] if et < 4 else kT[:, et - 4, :]
                            if has_qkv_b:
                                nc.scalar.activation(
                                    out=dst, in_=ps, func=AF.Identity,
                                    bias=bqkv_sb[l][:, et:et + 1], scale=1.0)
                            else:
                                nc.scalar.copy(out=dst, in_=ps)

                        # --- v token-major [c, e] with per-head ones column ---
                        v_ext = acts.tile([128, CT, H, 72], fp8, tag="v_ext",
                                          name="v_ext")
                        nc.vector.memset(v_ext[:, :, :, HD:HD + 1], 1.0)
                        for ct in range(CT):
                            ps = psA.tile([128, C], f32, tag="ps_a", name="ps_a")
                            for dt_ in range(DT):
                                nc.tensor.matmul(
                                    ps, lhsT=xT[:, dt_, ct * 128:(ct + 1) * 128],
                                    rhs=wqkv_sb[l][:, dt_, 2 * D:3 * D],
                                    start=(dt_ == 0), stop=(dt_ == DT - 1))
                            if has_qkv_b:
                                tv = lnp.tile([128, D], f32, tag="ln_t", name="tv")
                                nc.vector.tensor_tensor(out=tv, in0=ps, in1=vb_sb[l],
                                                        op=AL.add)
                                nc.scalar.copy(out=v_ext[:, ct, :, 0:HD], in_=tv)
                            else:
                                nc.scalar.copy(out=v_ext[:, ct, :, 0:HD], in_=ps)

                        # --- attention, software-pipelined across heads ---
                        # PE order: scores(h) ... scores(h+1), av(h), so the PE
                        # never sits behind ACT's exp in its own queue.
                        oT = acts.tile([128, DT, C], bf16, tag="oT", name="oT")
                        expTs = [None] * H     # live expT tiles per head
                        psOs = [None] * H      # live AV psum per head

                        def emit_scores(h):
                            th, off = h // 2, (h % 2) * 64
                            expT = expp.tile([128, CT, C], fp8, tag="expT",
                                             name="expT")
                            for kt in range(CT):
                                ps = psS.tile([128, C], f32, tag="ps_s", name="ps_s")
                                nc.tensor.matmul(
                                    ps,
                                    lhsT=kT[off:off + 64, th, kt * 128:(kt + 1) * 128],
                                    rhs=qT[off:off + 64, th, :], start=True, stop=True)
                                nc.scalar.activation(out=expT[:, kt, :], in_=ps,
                                                     func=AF.Exp, scale=1.0 / 8.0)
                            expTs[h] = expT

                        def emit_av(h):
                            pso = psO.tile([128, C], f32, tag="ps_o", name="ps_o")
                            for kp in range(CT // 2):
                                nc.tensor.matmul(
                                    pso[0:HD + 1, :],
                                    lhsT=v_ext[:, 2 * kp:2 * kp + 2, h, 0:HD + 1],
                                    rhs=expTs[h][:, 2 * kp:2 * kp + 2, :],
                                    start=(kp == 0), stop=(kp == CT // 2 - 1),
                                    perf_mode=DR)
                            psOs[h] = pso

                        def emit_norm(p):
                            # heads 2p (rows 0:64) and 2p+1 (rows 64:128)
                            h0, h1 = 2 * p, 2 * p + 1
                            th = p
                            # denominator rows to SBUF (custom DVE ops must
                            # not read PSUM), ONE fast reciprocal, bf16 cast,
                            # 1-cyc/row bf16 broadcast matmuls into the two
                            # partition halves, single ACT bounce to SBUF,
                            # then two PSUM-direct DVE mults.
                            den2 = nrm.tile([1, 2, C], f32, tag="den2",
                                            name="den2")
                            nc.vector.tensor_copy(den2[:, 0, :],
                                                  psOs[h0][HD:HD + 1, :])
                            nc.vector.tensor_copy(den2[:, 1, :],
                                                  psOs[h1][HD:HD + 1, :])
                            rec2 = nrm.tile([1, 2, C], f32, tag="rec2",
                                            name="rec2")
                            nc.vector.reciprocal_approx_fast(rec2, den2)
                            rec2b = nrm.tile([1, 2, C], bf16, tag="rec2b",
                                             name="rec2b")
                            nc.vector.tensor_copy(rec2b, rec2)
                            psd = psA.tile([128, C], f32, tag="ps_a", name="ps_d")
                            nc.tensor.matmul(
                                psd[0:64, :], lhsT=ones64b, rhs=rec2b[:, 0, :],
                                start=True, stop=True, skip_group_check=True)
                            nc.tensor.matmul(
                                psd[64:128, :], lhsT=ones64b, rhs=rec2b[:, 1, :],
                                start=True, stop=True, skip_group_check=True)
                            rec_bc = nrm.tile([128, C], f32, tag="rec_bc",
                                              name="rec_bc")
                            nc.scalar.copy(out=rec_bc, in_=psd)
                            nc.vector.tensor_tensor(
                                out=oT[0:64, th, :], in0=psOs[h0][0:HD, :],
                                in1=rec_bc[0:64, :], op=AL.mult)
                            nc.vector.tensor_tensor(
                                out=oT[64:128, th, :], in0=psOs[h1][0:HD, :],
                                in1=rec_bc[64:128, :], op=AL.mult)
                            psOs[h0] = psOs[h1] = None

                        emit_scores(0)
                        for h in range(1, H):
                            emit_scores(h)
                            emit_av(h - 1)
                            if h >= 2 and h % 2 == 0:
                                emit_norm((h - 2) // 2)
                        emit_av(H - 1)
                        emit_norm(H // 2 - 1)

                        # --- out-projection + residual + LN1 ---
                        xm2 = xmp.tile([128, CT, D], f32, tag="xm", name="xm2")
                        for ct in range(CT):
                            ps = psA.tile([128, C], f32, tag="ps_a", name="ps_a")
                            for et in range(DT):
                                nc.tensor.matmul(
                                    ps, lhsT=oT[:, et, ct * 128:(ct + 1) * 128],
                                    rhs=wo_sb[l][:, et, :],
                                    start=(et == 0), stop=(et == DT - 1))
                            if has_out_b:
                                nc.vector.tensor_tensor(out=ps, in0=ps, in1=outb_sb[l],
                                                        op=AL.add)
                            ln_block(ps, x_in[:, ct, :],
                                     ln1w_sb[l] if has_ln1 else None,
                                     ln1b_sb[l] if has_ln1 else None,
                                     xm2[:, ct, :])

                        # --- FFN ---
                        x2T = xtp.tile([128, DT, C], bf16, tag="xT", name="x2T")
                        transpose_to(xm2, x2T)
                        hT = acts.tile([128, FT, C], bf16, tag="hT", name="hT")
                        for ft in range(FT):
                            ps = psA.tile([128, C], f32, tag="ps_a", name="ps_a")
                            for dt_ in range(DT):
                                nc.tensor.matmul(
                                    ps, lhsT=w1_sb[l][:, dt_, ft * 128:(ft + 1) * 128],
                                    rhs=x2T[:, dt_, :],
                                    start=(dt_ == 0), stop=(dt_ == DT - 1))
                            nc.scalar.activation(
                                out=hT[:, ft, :], in_=ps, func=AF.Relu,
                                bias=(b1_sb[l][:, ft:ft + 1] if has_ff1_b else 0.0),
                                scale=1.0)
                        x_next = xmp.tile([128, CT, D], f32, tag="xm", name="x_next")
                        for ct in range(CT):
                            ps = psA.tile([128, C], f32, tag="ps_a", name="ps_a")
                            for ft in range(FT):
                                nc.tensor.matmul(
                                    ps, lhsT=hT[:, ft, ct * 128:(ct + 1) * 128],
                                    rhs=w2_sb[l][:, ft, :],
                                    start=(ft == 0), stop=(ft == FT - 1))
                            if has_ff2_b:
                                nc.vector.tensor_tensor(out=ps, in0=ps, in1=ff2b_sb[l],
                                                        op=AL.add)
                            ln_block(ps, xm2[:, ct, :],
                                     ln2w_sb[l] if has_ln2 else None,
                                     ln2b_sb[l] if has_ln2 else None,
                                     x_next[:, ct, :])
                        x_in = x_next

                    # ---------------- phase 3: final LN -> y_bf ----------------
                    for ct in range(CT):
                        ln_block(x_in[:, ct, :], None, flnw_sb, flnb_sb,
                                 y_bf[:, ct, :])

            # ============ scope: expand ============
            # out[s, :] = y[seg[s], :] via one-hot^T matmuls; bf16 output in
            # 1 MiB group DMAs.
            with (
                tc.tile_pool(name="ohp", bufs=2 * GT) as ohp,
                tc.tile_pool(name="outp", bufs=3) as outp,
                tc.tile_pool(name="psE", bufs=4, space="PSUM") as psE,
            ):
                seg_row_ap = seg_row[:, :]
                for g in range(NG):
                    if g == 0:
                        seg_bc = seg_bc0
                    else:
                        seg_bc = ohp.tile([128, GT * 128], f16, tag="seg_bc",
                                          name="seg_bc")
                        src = bass.AP(tensor=seg_row_ap.tensor,
                                      offset=g * GT * 128,
                                      ap=[[0, 128], [1, GT * 128]])
                        nc.gpsimd.dma_start(out=seg_bc, in_=src)
                    og = outp.tile([128, GT, D], bf16, tag="og", name="og")
                    ohTs = []
                    for j in range(GT):
                        t = g * GT + j
                        lo, hi = ranges[t]
                        ohT = ohp.tile([128, CT, 128], bf16, tag="ohT", name="ohT")
                        for m in range(lo, hi + 1):
                            nc.vector.tensor_scalar(
                                out=ohT[:, m, :],
                                in0=seg_bc[:, j * 128:(j + 1) * 128],
                                scalar1=iota_col_sb[:, m:m + 1], scalar2=None,
                                op0=AL.is_equal)
                        ohTs.append(ohT)
                    for j in range(GT):
                        t = g * GT + j
                        lo, hi = ranges[t]
                        pse = psE.tile([128, D], f32, tag="ps_e", name="ps_e")
                        for m in range(lo, hi + 1):
                            nc.tensor.matmul(
                                pse, lhsT=ohTs[j][:, m, :],
                                rhs=y_bf[:, m, :],
                                start=(m == lo), stop=(m == hi))
                        if t % 4 == 3:
                            nc.vector.tensor_copy(og[:, j, :], pse)
                        else:
                            nc.scalar.copy(out=og[:, j, :], in_=pse)
                    nc.sync.dma_start(
                        out=out_d[g * GT * 128:(g + 1) * GT * 128, :]
                        .rearrange("(n p) d -> p n d", p=128),
                        in_=og)

    return nc


def _host_prep(inputs):
    """Shard + preprocess full inputs into 8 per-core input maps."""
    bf = ml_dtypes.bfloat16
    f8 = ml_dtypes.float8_e4m3fn
    tokens = np.asarray(inputs["tokens"], dtype=np.float32)
    seg = np.asarray(inputs["segment_ids"], dtype=np.int32)
    qkv_w = np.asarray(inputs["qkv_w"], dtype=np.float32)
    qkv_b = np.asarray(inputs["qkv_b"], dtype=np.float32)
    out_w = np.asarray(inputs["out_w"], dtype=np.float32)
    out_b = np.asarray(inputs["out_b"], dtype=np.float32)
    ln1_w = np.asarray(inputs["ln1_w"], dtype=np.float32)
    ln1_b = np.asarray(inputs["ln1_b"], dtype=np.float32)
    ln2_w = np.asarray(inputs["ln2_w"], dtype=np.float32)
    ln2_b = np.asarray(inputs["ln2_b"], dtype=np.float32)
    ff1_w = np.asarray(inputs["ff1_w"], dtype=np.float32)
    ff1_b = np.asarray(inputs["ff1_b"], dtype=np.float32)
    ff2_w = np.asarray(inputs["ff2_w"], dtype=np.float32)
    ff2_b = np.asarray(inputs["ff2_b"], dtype=np.float32)
    fln_w = np.asarray(inputs["fln_w"], dtype=np.float32)
    fln_b = np.asarray(inputs["fln_b"], dtype=np.float32)

    flags = (
        bool(np.any(qkv_b)),
        bool(np.any(out_b)),
        bool(np.any(ff1_b)),
        bool(np.any(ff2_b)),
        bool(np.any(ln1_w != 1.0) or np.any(ln1_b)),
        bool(np.any(ln2_w != 1.0) or np.any(ln2_b)),
        bool(np.any(fln_w != 1.0) or np.any(fln_b)),
    )

    # span-bound ranges: per token tile, union over batch of the contiguous
    # chunk-tile range its (sorted) segment ids cover.
    srt = np.all(np.diff(seg, axis=1) >= 0)
    if srt:
        lo = np.min(seg[:, ::128] // 128, axis=0)
        hi = np.max(seg[:, 127::128] // 128, axis=0)
    else:  # fallback: no structure assumed
        lo = np.zeros(NT, np.int64)
        hi = np.full(NT, CT - 1, np.int64)
    covered = set()
    for t in range(NT):
        covered.update(range(int(lo[t]), int(hi[t]) + 1))
    if covered != set(range(CT)):
        lo = np.zeros(NT, np.int64)
        hi = np.full(NT, CT - 1, np.int64)
    ranges = tuple((int(lo[t]), int(hi[t])) for t in range(NT))

    # shared (batch-independent) arrays
    shared = {
        "iota_row": np.broadcast_to(
            np.arange(C, dtype=np.float16)[None, :], (128, C)).copy(),
        "iota_col": (np.arange(CT, dtype=np.float32)[None, :] * 128
                     + np.arange(128, dtype=np.float32)[:, None]).astype(np.float32),
        "wqkvT": np.ascontiguousarray(qkv_w.transpose(0, 2, 1)).astype(bf),
        "woT": np.ascontiguousarray(out_w.transpose(0, 2, 1)).astype(bf),
        "w1T": np.ascontiguousarray(ff1_w.transpose(0, 2, 1)).astype(bf),
        "w2T": np.ascontiguousarray(ff2_w.transpose(0, 2, 1)).astype(bf),
    }
    (has_qkv_b, has_out_b, has_ff1_b, has_ff2_b,
     has_ln1, has_ln2, has_fln) = flags
    if has_qkv_b:
        shared["bqkv_c"] = np.ascontiguousarray(
            qkv_b[:, :1536].reshape(L, 12, 128).transpose(0, 2, 1))
        shared["vb_row"] = np.ascontiguousarray(qkv_b[:, 2 * D:3 * D][:, None, :])
    if has_ff1_b:
        shared["b1_c"] = np.ascontiguousarray(
            ff1_b.reshape(L, FT, 128).transpose(0, 2, 1))
    if has_out_b:
        shared["outb_row"] = np.ascontiguousarray(out_b[:, None, :])
    if has_ff2_b:
        shared["ff2b_row"] = np.ascontiguousarray(ff2_b[:, None, :])
    if has_ln1:
        shared["ln1w_row"] = np.ascontiguousarray(ln1_w[:, None, :])
        shared["ln1b_row"] = np.ascontiguousarray(ln1_b[:, None, :])
    if has_ln2:
        shared["ln2w_row"] = np.ascontiguousarray(ln2_w[:, None, :])
        shared["ln2b_row"] = np.ascontiguousarray(ln2_b[:, None, :])
    if has_fln:
        shared["flnw_row"] = np.ascontiguousarray(fln_w[None, :])
        shared["flnb_row"] = np.ascontiguousarray(fln_b[None, :])

    in_maps = []
    for b in range(B):
        m = dict(shared)
        m["tokens_bf"] = np.ascontiguousarray(tokens[b]).astype(bf)
        m["seg_col"] = np.ascontiguousarray(
            seg[b].reshape(NT, 128).T.astype(np.float32))
        m["seg_row"] = np.ascontiguousarray(seg[b].astype(np.float16)[None, :])
        counts = np.bincount(seg[b], minlength=C).astype(np.float32)
        m["inv_cnt"] = np.ascontiguousarray(
            (1.0 / np.maximum(counts, 1.0)).reshape(CT, 128).T)
        in_maps.append(m)
    return flags, ranges, in_maps


def kernel(**inputs) -> np.ndarray:
    from concourse.bass_utils import run_bass_kernel_spmd

    flags, ranges, in_maps = _host_prep(inputs)
    key = (flags, ranges)
    if key not in _CACHE:
        nc = _build(flags, ranges)
        if not nc.is_finalized():
            nc.finalize()
        _CACHE[key] = nc
    nc = _CACHE[key]
    res = run_bass_kernel_spmd(nc, in_maps, list(range(B)))
    return np.stack(
        [np.asarray(res.results[i]["out_bf"]).astype(np.float32)
         for i in range(B)], axis=0)


# revision 37
# speedup vs baseline: 1.1489x; 1.0166x over previous
"""Trainium2 Bass kernel for nn_ChunkProcessor (segment-mean -> 2-layer
transformer encoder over chunks -> gather-expand -> final LN).

Sharding: data-parallel over batch B=8 across the 8 NeuronCores; each core
processes one batch item end to end (no cross-core communication).

v2 design notes (perf):
  - tokens / weights pre-cast to bf16 on HOST; output written bf16 and
    upcast on host: halves all big HBM traffic.
  - 1/counts computed on host -> no count matmuls, no phase-1 reciprocals,
    PSUM banks freed.
  - tokens + output move in 1 MiB group DMAs (8 tiles each) for full DMA BW.
  - attention softmax normalization: denominators from the v ones-column,
    reciprocal_approx_fast on a [2,C] pair tile, broadcast to 128 partitions
    with ONE fp32r matmul per head pair (1 cyc/row), numerators bounced to
    SBUF on ACT, one DVE mult per head. Kills the [1,C] exact reciprocals
    (1.4us each) and fp32 1x64 broadcast matmuls (1us each) of v1.
  - attention software-pipelined: scores for head h+1 are emitted on PE
    before AV of head h so the PE never waits on ACT's exp -- keeps the PE
    HAM clock gate at 2.4 GHz (idle PE re-throttles to 1.2 GHz).
"""

import numpy as np
import ml_dtypes

B, S, D = 8, 8192, 512
C, H, L, DFF = 512, 8, 2, 2048
HD = D // H          # 64
NT = S // 128        # 64 token tiles
CT = C // 128        # 4 chunk tiles
DT = D // 128        # 4 feature tiles
FT = DFF // 128      # 16
GT = 8               # token tiles per DMA group
NG = NT // GT        # token-tile DMA groups
SW = 64.0            # fp8 weight pre-scale (host); folded back as 1/SW
EPS = 1e-5

_CACHE = {}


def _build(flags, ranges):
    """Build the Bass program.

    flags  = (qkv_b, out_b, ff1_b, ff2_b, ln1_aff, ln2_aff, fln_aff) bools.
    ranges = tuple of (lo_m, hi_m) per token tile t: the contiguous range of
             chunk tiles any batch item's tile-t segment ids fall into.
    """
    import concourse.bass as bass
    import concourse.tile as tile
    from concourse import bacc, mybir
    from concourse.masks import make_identity

    (has_qkv_b, has_out_b, has_ff1_b, has_ff2_b,
     has_ln1, has_ln2, has_fln) = flags

    # first/last contributing token tile per chunk tile (for PSUM start/stop)
    first_t = [min(t for t in range(NT) if ranges[t][0] <= m <= ranges[t][1])
               for m in range(CT)]
    last_t = [max(t for t in range(NT) if ranges[t][0] <= m <= ranges[t][1])
              for m in range(CT)]

    f32 = mybir.dt.float32
    f32r = mybir.dt.float32r
    bf16 = mybir.dt.bfloat16
    f16 = mybir.dt.float16
    fp8 = mybir.dt.float8e4
    AL = mybir.AluOpType
    AF = mybir.ActivationFunctionType
    DR = mybir.MatmulPerfMode.DoubleRow

    nc = bacc.Bacc("TRN2", target_bir_lowering=False)

    tokens = nc.declare_dram_parameter("tokens_bf", [S, D], bf16, isOutput=False)
    seg_col = nc.declare_dram_parameter("seg_col", [128, NT], f32, isOutput=False)
    seg_row = nc.declare_dram_parameter("seg_row", [1, S], f16, isOutput=False)
    iota_row = nc.declare_dram_parameter("iota_row", [128, C], f16, isOutput=False)
    iota_col = nc.declare_dram_parameter("iota_col", [128, CT], f32, isOutput=False)
    inv_cnt = nc.declare_dram_parameter("inv_cnt", [128, CT], f32, isOutput=False)
    wqkvT = nc.declare_dram_parameter("wqkvT", [L, D, 3 * D], bf16, isOutput=False)
    woT = nc.declare_dram_parameter("woT", [L, D, D], bf16, isOutput=False)
    w1T = nc.declare_dram_parameter("w1T", [L, D, DFF], bf16, isOutput=False)
    w2T = nc.declare_dram_parameter("w2T", [L, DFF, D], bf16, isOutput=False)
    if has_qkv_b:
        bqkv_c = nc.declare_dram_parameter("bqkv_c", [L, 128, 12], f32, isOutput=False)
        vb_row = nc.declare_dram_parameter("vb_row", [L, 1, D], f32, isOutput=False)
    if has_ff1_b:
        b1_c = nc.declare_dram_parameter("b1_c", [L, 128, FT], f32, isOutput=False)
    if has_out_b:
        outb_row = nc.declare_dram_parameter("outb_row", [L, 1, D], f32, isOutput=False)
    if has_ff2_b:
        ff2b_row = nc.declare_dram_parameter("ff2b_row", [L, 1, D], f32, isOutput=False)
    if has_ln1:
        ln1w_row = nc.declare_dram_parameter("ln1w_row", [L, 1, D], f32, isOutput=False)
        ln1b_row = nc.declare_dram_parameter("ln1b_row", [L, 1, D], f32, isOutput=False)
    if has_ln2:
        ln2w_row = nc.declare_dram_parameter("ln2w_row", [L, 1, D], f32, isOutput=False)
        ln2b_row = nc.declare_dram_parameter("ln2b_row", [L, 1, D], f32, isOutput=False)
    if has_fln:
        flnw_row = nc.declare_dram_parameter("flnw_row", [1, D], f32, isOutput=False)
        flnb_row = nc.declare_dram_parameter("flnb_row", [1, D], f32, isOutput=False)
    out_d = nc.declare_dram_parameter("out_bf", [S, D], bf16, isOutput=True)

    def bcast_load(pool, dram_row, tag):
        """DMA a [1, D] DRAM row into a [128, D] SBUF tile (partition bcast)."""
        t = pool.tile([128, D], f32, tag=tag, name=f"row_{tag}")
        src = bass.AP(tensor=dram_row.tensor, offset=dram_row.offset,
                      ap=[[0, 128]] + [list(p) for p in dram_row.ap[1:]])
        nc.gpsimd.dma_start(out=t, in_=src)
        return t

    with tile.TileContext(nc) as tc:
        with (
            tc.tile_pool(name="consts", bufs=1) as consts,
            tc.tile_pool(name="acts", bufs=1) as acts,
            tc.tile_pool(name="xm", bufs=2) as xmp,
            tc.tile_pool(name="xt", bufs=2) as xtp,
            tc.tile_pool(name="lnp", bufs=2) as lnp,
            tc.tile_pool(name="nrm", bufs=1) as nrm,
            tc.tile_pool(name="rows", bufs=1) as rows,
        ):
            # ---------------- constants ----------------
            seg_col_sb = consts.tile([128, NT], f32)
            nc.gpsimd.dma_start(out=seg_col_sb, in_=seg_col[:, :])
            iota_row_sb = consts.tile([128, C], f16)
            nc.gpsimd.dma_start(out=iota_row_sb, in_=iota_row[:, :])
            iota_col_sb = consts.tile([128, CT], f32)
            nc.gpsimd.dma_start(out=iota_col_sb, in_=iota_col[:, :])
            inv_cnt_sb = consts.tile([128, CT], f32)
            nc.gpsimd.dma_start(out=inv_cnt_sb, in_=inv_cnt[:, :])
            ones64b = consts.tile([1, 64], bf16)
            nc.vector.memset(ones64b, 1.0)
            ones_row32 = consts.tile([1, 128], f32)
            nc.vector.memset(ones_row32, 1.0)
            ident32 = consts.tile([128, 128], f32)
            make_identity(nc, ident32)
            eps_t = consts.tile([128, 1], f32)
            nc.vector.memset(eps_t, EPS)

            # y_bf lives in the persistent pool (used by the expand phase
            # after the weight pool is closed).
            y_bf = acts.tile([128, CT, D], bf16, tag="y_bf")
            # group-0 expand one-hot source, prefetched during phase 2 so the
            # expand phase does not start with a cold DMA latency chain.
            seg_bc0 = acts.tile([128, GT * 128], f16, tag="seg_bc0")
            nc.gpsimd.dma_start(
                out=seg_bc0,
                in_=bass.AP(tensor=seg_row[:, :].tensor, offset=0,
                            ap=[[0, 128], [1, GT * 128]]))

            def ln_block(ps_src, resid, wrow, brow, out_ap, pre_scale=None):
                # out = LN(ps_src * pre_scale + resid) [* w + b]  (token-major)
                t_ = lnp.tile([128, D], f32, tag="ln_t", name="ln_t")
                if resid is not None and pre_scale is not None:
                    nc.vector.scalar_tensor_tensor(
                        out=t_, in0=ps_src, scalar=pre_scale, in1=resid,
                        op0=AL.mult, op1=AL.add)
                elif resid is not None:
                    nc.vector.tensor_tensor(out=t_, in0=ps_src, in1=resid, op=AL.add)
                else:
                    nc.vector.tensor_copy(t_, ps_src)
                st = lnp.tile([128, 6], f32, tag="ln_st", name="ln_st")
                nc.vector.bn_stats(out=st, in_=t_)
                mv = lnp.tile([128, 2], f32, tag="ln_mv", name="ln_mv")
                nc.vector.bn_aggr(out=mv, in_=st)
                sd = lnp.tile([128, 1], f32, tag="ln_sd", name="ln_sd")
                nc.scalar.activation(out=sd, in_=mv[:, 1:2], func=AF.Sqrt,
                                     bias=eps_t[:, 0:1], scale=1.0)
                rs = lnp.tile([128, 1], f32, tag="ln_rs", name="ln_rs")
                nc.vector.reciprocal(rs, sd)
                if wrow is None:
                    nc.vector.tensor_scalar(
                        out=out_ap, in0=t_, scalar1=mv[:, 0:1], scalar2=rs[:, 0:1],
                        op0=AL.subtract, op1=AL.mult)
                else:
                    xn = lnp.tile([128, D], f32, tag="ln_xn", name="ln_xn")
                    nc.vector.tensor_scalar(
                        out=xn, in0=t_, scalar1=mv[:, 0:1], scalar2=rs[:, 0:1],
                        op0=AL.subtract, op1=AL.mult)
                    nc.vector.tensor_tensor(out=xn, in0=xn, in1=wrow, op=AL.mult)
                    nc.vector.tensor_tensor(out=out_ap, in0=xn, in1=brow, op=AL.add)

            # ============ scope: weights + segsum + transformer ============
            with (
                tc.tile_pool(name="wts", bufs=1) as wts,
                tc.tile_pool(name="expp", bufs=2) as expp,
            ):
                # ---- weights (bf16 in DRAM already) ----
                # All big DMAs share the ONE sync HWDGE ring so ring order =
                # transfer order: an up-front 12MB weight prefetch on its own
                # queue starves the phase-1 token stream (SDMA round-robins
                # between queues; measured: first segsum matmul at 35us).
                # Weight loads are emitted mid-phase-1 instead (see below).
                wqkv_sb = [wts.tile([128, DT, 3 * D], bf16, tag=f"wqkv{l}",
                                    name=f"wqkv{l}") for l in range(L)]
                wo_sb = [wts.tile([128, DT, D], bf16, tag=f"wo{l}",
                                  name=f"wo{l}") for l in range(L)]
                w1_sb = [wts.tile([128, DT, DFF], bf16, tag=f"w1{l}",
                                  name=f"w1{l}") for l in range(L)]
                w2_sb = [wts.tile([128, FT, D], bf16, tag=f"w2{l}",
                                  name=f"w2{l}") for l in range(L)]

                def load_weights(l):
                    nc.sync.dma_start(
                        out=wqkv_sb[l],
                        in_=wqkvT[l].rearrange("(dt p) e -> p dt e", p=128))
                    nc.sync.dma_start(
                        out=wo_sb[l],
                        in_=woT[l].rearrange("(dt p) e -> p dt e", p=128))
                    nc.sync.dma_start(
                        out=w1_sb[l],
                        in_=w1T[l].rearrange("(dt p) e -> p dt e", p=128))
                    nc.sync.dma_start(
                        out=w2_sb[l],
                        in_=w2T[l].rearrange("(ft p) e -> p ft e", p=128))

                bqkv_sb, b1_sb = [], []
                vb_sb, outb_sb, ff2b_sb = [], [], []
                ln1w_sb, ln1b_sb, ln2w_sb, ln2b_sb = [], [], [], []
                for l in range(L):
                    if has_qkv_b:
                        bq = consts.tile([128, 12], f32, tag=f"bqkv{l}", name=f"bqkv{l}")
                        nc.sync.dma_start(out=bq, in_=bqkv_c[l])
                        bqkv_sb.append(bq)
                        vb_sb.append(bcast_load(rows, vb_row[l], f"vb{l}"))
                    if has_ff1_b:
                        b1 = consts.tile([128, FT], f32, tag=f"b1{l}", name=f"b1{l}")
                        nc.sync.dma_start(out=b1, in_=b1_c[l])
                        b1_sb.append(b1)
                    if has_out_b:
                        outb_sb.append(bcast_load(rows, outb_row[l], f"outb{l}"))
                    if has_ff2_b:
                        ff2b_sb.append(bcast_load(rows, ff2b_row[l], f"ff2b{l}"))
                    if has_ln1:
                        ln1w_sb.append(bcast_load(rows, ln1w_row[l], f"ln1w{l}"))
                        ln1b_sb.append(bcast_load(rows, ln1b_row[l], f"ln1b{l}"))
                    if has_ln2:
                        ln2w_sb.append(bcast_load(rows, ln2w_row[l], f"ln2w{l}"))
                        ln2b_sb.append(bcast_load(rows, ln2b_row[l], f"ln2b{l}"))
                flnw_sb = bcast_load(rows, flnw_row, "flnw") if has_fln else None
                flnb_sb = bcast_load(rows, flnb_row, "flnb") if has_fln else None

                # ------------ phase 1: segment sums -> means ------------
                # bf16 token tiles stream over HWDGE in 1 MiB groups of 8;
                # one-hot matmuls accumulate sums in PSUM; host-computed
                # 1/counts turns them into means (no count matmuls).
                x0 = xmp.tile([128, CT, D], f32, tag="xm", name="x0")
                with (
                    tc.tile_pool(name="pseg", bufs=1, space="PSUM") as pseg,
                    tc.tile_pool(name="segs", bufs=2) as segs,
                    tc.tile_pool(name="ohp1", bufs=3) as ohp1,
                ):
                    ps_sums = [pseg.tile([128, D], f32, tag=f"sums{m}", name=f"sums{m}")
                               for m in range(CT)]
                    for g in range(NG):
                        tokg = segs.tile([128, GT, D], bf16, tag="tokg", name="tokg")
                        h = GT // 2
                        for hf in range(2):
                            base = g * GT + hf * h
                            nc.sync.dma_start(
                                out=tokg[:, hf * h:(hf + 1) * h, :],
                                in_=tokens[base * 128:(base + h) * 128, :]
                                .rearrange("(n p) d -> p n d", p=128))
                        for j in range(GT):
                            t = g * GT + j
                            lo, hi = ranges[t]
                            oh = ohp1.tile([128, C], bf16, tag="oh", name="oh")
                            sl = slice(lo * 128, (hi + 1) * 128)
                            nc.vector.tensor_scalar(
                                out=oh[:, sl], in0=iota_row_sb[:, sl],
                                scalar1=seg_col_sb[:, t:t + 1],
                                scalar2=None, op0=AL.is_equal)
                            for m in range(lo, hi + 1):
                                nc.tensor.matmul(
                                    ps_sums[m], lhsT=oh[:, m * 128:(m + 1) * 128],
                                    rhs=tokg[:, j, :],
                                    start=(t == first_t[m]), stop=(t == last_t[m]))
                        if g >= NG - 4:
                            # Ghost WAW dep: the tiny copy into the weight
                            # tile pins this qkv0 chunk AFTER group-g tokens
                            # in the scheduler (emission order alone is just
                            # a priority hint and gets hoisted). One 384KB
                            # dt-chunk rides between token groups so tokens
                            # never wait behind a monolithic 1.5MB load.
                            dt_c = g - (NG - 4)
                            nc.vector.tensor_copy(
                                wqkv_sb[0][0:1, dt_c, 0:1], tokg[0:1, 0, 0:1])
                            nc.sync.dma_start(
                                out=wqkv_sb[0][:, dt_c, :],
                                in_=wqkvT[0][dt_c * 128:(dt_c + 1) * 128, :]
                                .rearrange("p e -> p e"))
                    # x = sums * (1/count)
                    for m in range(CT):
                        nc.vector.tensor_scalar(
                            out=x0[:, m, :], in0=ps_sums[m],
                            scalar1=inv_cnt_sb[:, m:m + 1],
                            scalar2=None, op0=AL.mult)
                    # remaining weights stream during phase-2 compute, in ring
                    # order, ghost-dep'd on x0 so they cannot be hoisted into
                    # the token stream.
                    for wtile in (wo_sb[0], w1_sb[0], wqkv_sb[1], wo_sb[1],
                                  w1_sb[1]):
                        nc.vector.tensor_copy(wtile[0:1, 0, 0:1],
                                              x0[0:1, 0, 0:1])
                    for wtile in (w2_sb[0], w2_sb[1]):
                        nc.vector.tensor_copy(wtile[0:1, 0, 0:1],
                                              x0[0:1, 0, 0:1])
                    nc.sync.dma_start(
                        out=wo_sb[0],
                        in_=woT[0].rearrange("(dt p) e -> p dt e", p=128))
                    nc.sync.dma_start(
                        out=w1_sb[0],
                        in_=w1T[0].rearrange("(dt p) e -> p dt e", p=128))
                    nc.sync.dma_start(
                        out=w2_sb[0],
                        in_=w2T[0].rearrange("(ft p) e -> p ft e", p=128))
                    load_weights(1)

                # ---------------- phase 2: transformer ----------------
                with (
                    tc.tile_pool(name="psA", bufs=2, space="PSUM") as psA,
                    tc.tile_pool(name="psS", bufs=2, space="PSUM") as psS,
                    tc.tile_pool(name="psO", bufs=2, space="PSUM") as psO,
                ):
                    def transpose_to(src_f32, dst_bf16):
                        # src: [128, CT, D] f32 token-major; dst: [128, DT, C] bf16
                        for i in range(CT):
                            for j in range(DT):
                                pst = psS.tile([128, 128], f32, tag="ps_t", name="ps_t")
                                nc.tensor.transpose(
                                    pst, src_f32[:, i, j * 128:(j + 1) * 128], ident32)
                                if (i + j) % 2 == 0:
                                    nc.scalar.copy(
                                        out=dst_bf16[:, j, i * 128:(i + 1) * 128],
                                        in_=pst)
                                else:
                                    nc.vector.tensor_copy(
                                        dst_bf16[:, j, i * 128:(i + 1) * 128],
                                        pst)

                    x_in = x0
                    for l in range(L):
                        xT = xtp.tile([128, DT, C], bf16, tag="xT", name="xT")
                        transpose_to(x_in, xT)

                        # --- q, k feature-major [e, c] ---
                        qT = acts.tile([128, DT, C], bf16, tag="qT", name="qT")
                        kT = acts.tile([128, DT, C], bf16, tag="kT", name="kT")
                        for et in range(8):
                            ps = psA.tile([128, C], f32, tag="ps_a", name="ps_a")
                            for dt_ in range(DT):
                                nc.tensor.matmul(
                                    ps, lhsT=wqkv_sb[l][:, dt_, et * 128:(et + 1) * 128],
                                    rhs=xT[:, dt_, :],
                                    start=(dt_ == 0), stop=(dt_ == DT - 1))
                            dst = qT[:, et, :] if et < 4 else kT[:, et - 4, :]
                            if has_qkv_b:
                                nc.scalar.activation(
                                    out=dst, in_=ps, func=AF.Identity,
                                    bias=bqkv_sb[l][:, et:et + 1], scale=1.0)
                            else:
                                nc.scalar.copy(out=dst, in_=ps)

                        # --- v token-major [c, e] with per-head ones column ---
                        v_ext = acts.tile([128, CT, H, 72], fp8, tag="v_ext",
                                          name="v_ext")
                        nc.vector.memset(v_ext[:, :, :, HD:HD + 1], 1.0)
                        for ct in range(CT):
                            ps = psA.tile([128, C], f32, tag="ps_a", name="ps_a")
                            for dt_ in range(DT):
                                nc.tensor.matmul(
                                    ps, lhsT=xT[:, dt_, ct * 128:(ct + 1) * 128],
                                    rhs=wqkv_sb[l][:, dt_, 2 * D:3 * D],
                                    start=(dt_ == 0), stop=(dt_ == DT - 1))
                            if has_qkv_b:
                                tv = lnp.tile([128, D], f32, tag="ln_t", name="tv")
                                nc.vector.tensor_tensor(out=tv, in0=ps, in1=vb_sb[l],
                                                        op=AL.add)
                                nc.scalar.copy(out=v_ext[:, ct, :, 0:HD], in_=tv)
                            else:
                                nc.scalar.copy(out=v_ext[:, ct, :, 0:HD], in_=ps)

                        # --- attention, software-pipelined across heads ---
                        # PE order: scores(h) ... scores(h+1), av(h), so the PE
                        # never sits behind ACT's exp in its own queue.
                        oT = acts.tile([128, DT, C], bf16, tag="oT", name="oT")
                        expTs = [None] * H     # live expT tiles per head
                        psOs = [None] * H      # live AV psum per head

                        def emit_scores(h):
                            th, off = h // 2, (h % 2) * 64
                            expT = expp.tile([128, CT, C], fp8, tag="expT",
                                             name="expT")
                            for kt in range(CT):
                                ps = psS.tile([128, C], f32, tag="ps_s", name="ps_s")
                                nc.tensor.matmul(
                                    ps,
                                    lhsT=kT[off:off + 64, th, kt * 128:(kt + 1) * 128],
                                    rhs=qT[off:off + 64, th, :], start=True, stop=True)
                                nc.scalar.activation(out=expT[:, kt, :], in_=ps,
                                                     func=AF.Exp, scale=1.0 / 8.0)
                            expTs[h] = expT

                        def emit_av(h):
                            pso = psO.tile([128, C], f32, tag="ps_o", name="ps_o")
                            for kp in range(CT // 2):
                                nc.tensor.matmul(
                                    pso[0:HD + 1, :],
                                    lhsT=v_ext[:, 2 * kp:2 * kp + 2, h, 0:HD + 1],
                                    rhs=expTs[h][:, 2 * kp:2 * kp + 2, :],
                                    start=(kp == 0), stop=(kp == CT // 2 - 1),
                                    perf_mode=DR)
                            psOs[h] = pso

                        def emit_norm(p):
                            # heads 2p (rows 0:64) and 2p+1 (rows 64:128)
                            h0, h1 = 2 * p, 2 * p + 1
                            th = p
                            # denominator rows to SBUF (custom DVE ops must
                            # not read PSUM), ONE fast reciprocal, bf16 cast,
                            # 1-cyc/row bf16 broadcast matmuls into the two
                            # partition halves, single ACT bounce to SBUF,
                            # then two PSUM-direct DVE mults.
                            den2 = nrm.tile([1, 2, C], f32, tag="den2",
                                            name="den2")
                            nc.vector.tensor_copy(den2[:, 0, :],
                                                  psOs[h0][HD:HD + 1, :])
                            nc.vector.tensor_copy(den2[:, 1, :],
                                                  psOs[h1][HD:HD + 1, :])
                            rec2 = nrm.tile([1, 2, C], f32, tag="rec2",
                                            name="rec2")
                            nc.vector.reciprocal_approx_fast(rec2, den2)
                            rec2b = nrm.tile([1, 2, C], bf16, tag="rec2b",
                                             name="rec2b")
                            nc.vector.tensor_copy(rec2b, rec2)
                            psd = psA.tile([128, C], f32, tag="ps_a", name="ps_d")
                            nc.tensor.matmul(
                                psd[0:64, :], lhsT=ones64b, rhs=rec2b[:, 0, :],
                                start=True, stop=True, skip_group_check=True)
                            nc.tensor.matmul(
                                psd[64:128, :], lhsT=ones64b, rhs=rec2b[:, 1, :],
                                start=True, stop=True, skip_group_check=True)
                            rec_bc = nrm.tile([128, C], f32, tag="rec_bc",
                                              name="rec_bc")
                            nc.scalar.copy(out=rec_bc, in_=psd)
                            nc.vector.tensor_tensor(
                                out=oT[0:64, th, :], in0=psOs[h0][0:HD, :],
                                in1=rec_bc[0:64, :], op=AL.mult)
                            nc.vector.tensor_tensor(
                                out=oT[64:128, th, :], in0=psOs[h1][0:HD, :],
                                in1=rec_bc[64:128, :], op=AL.mult)
                            psOs[h0] = psOs[h1] = None

                        emit_scores(0)
                        for h in range(1, H):
                            emit_scores(h)
                            emit_av(h - 1)
                            if h >= 2 and h % 2 == 0:
                                emit_norm((h - 2) // 2)
                        emit_av(H - 1)
                        emit_norm(H // 2 - 1)

                        # --- out-projection + residual + LN1 ---
                        xm2 = xmp.tile([128, CT, D], f32, tag="xm", name="xm2")
                        for ct in range(CT):
                            ps = psA.tile([128, C], f32, tag="ps_a", name="ps_a")
                            for et in range(DT):
                                nc.tensor.matmul(
                                    ps, lhsT=oT[:, et, ct * 128:(ct + 1) * 128],
                                    rhs=wo_sb[l][:, et, :],
                                    start=(et == 0), stop=(et == DT - 1))
                            if has_out_b:
                                nc.vector.tensor_tensor(out=ps, in0=ps, in1=outb_sb[l],
                                                        op=AL.add)
                            ln_block(ps, x_in[:, ct, :],
                                     ln1w_sb[l] if has_ln1 else None,
                                     ln1b_sb[l] if has_ln1 else None,
                                     xm2[:, ct, :])

                        # --- FFN ---
                        x2T = xtp.tile([128, DT, C], bf16, tag="xT", name="x2T")
                        transpose_to(xm2, x2T)
                        hT = acts.tile([128, FT, C], bf16, tag="hT", name="hT")
                        for ft in range(FT):
                            ps = psA.tile([128, C], f32, tag="ps_a", name="ps_a")
                            for dt_ in range(DT):
                                nc.tensor.matmul(
                                    ps, lhsT=w1_sb[l][:, dt_, ft * 128:(ft + 1) * 128],
                                    rhs=x2T[:, dt_, :],
                                    start=(dt_ == 0), stop=(dt_ == DT - 1))
                            nc.scalar.activation(
                                out=hT[:, ft, :], in_=ps, func=AF.Relu,
                                bias=(b1_sb[l][:, ft:ft + 1] if has_ff1_b else 0.0),
                                scale=1.0)
                        x_next = xmp.tile([128, CT, D], f32, tag="xm", name="x_next")
                        for ct in range(CT):
                            ps = psA.tile([128, C], f32, tag="ps_a", name="ps_a")
                            for ft in range(FT):
                                nc.tensor.matmul(
                                    ps, lhsT=hT[:, ft, ct * 128:(ct + 1) * 128],
                                    rhs=w2_sb[l][:, ft, :],
                                    start=(ft == 0), stop=(ft == FT - 1))
                            if has_ff2_b:
                                nc.vector.tensor_tensor(out=ps, in0=ps, in1=ff2b_sb[l],
                                                        op=AL.add)
                            ln_block(ps, xm2[:, ct, :],
                                     ln2w_sb[l] if has_ln2 else None,
                                     ln2b_sb[l] if has_ln2 else None,
                                     x_next[:, ct, :])
                        x_in = x_next

                    # ---------------- phase 3: final LN -> y_bf ----------------
                    for ct in range(CT):
                        ln_block(x_in[:, ct, :], None, flnw_sb, flnb_sb,
                                 y_bf[:, ct, :])

            # ============ scope: expand ============
            # out[s, :] = y[seg[s], :] via one-hot^T matmuls; bf16 output in
            # 1 MiB group DMAs.
            with (
                tc.tile_pool(name="ohp", bufs=2 * GT + 2) as ohp,
                tc.tile_pool(name="outp", bufs=3) as outp,
                tc.tile_pool(name="psE", bufs=4, space="PSUM") as psE,
            ):
                seg_row_ap = seg_row[:, :]

                def make_ohts(g):
                    if g == 0:
                        seg_bc = seg_bc0
                    else:
                        seg_bc = ohp.tile([128, GT * 128], f16, tag="seg_bc",
                                          name="seg_bc")
                        ap = bass.AP(tensor=seg_row_ap.tensor,
                                     offset=g * GT * 128,
                                     ap=[[0, 128], [1, GT * 128]])
                        nc.gpsimd.dma_start(out=seg_bc, in_=ap)
                    ohTs = []
                    for j in range(GT):
                        t = g * GT + j
                        lo, hi = ranges[t]
                        ohT = ohp.tile([128, CT, 128], bf16, tag="ohT",
                                       name="ohT")
                        for m in range(lo, hi + 1):
                            nc.vector.tensor_scalar(
                                out=ohT[:, m, :],
                                in0=seg_bc[:, j * 128:(j + 1) * 128],
                                scalar1=iota_col_sb[:, m:m + 1], scalar2=None,
                                op0=AL.is_equal)
                        ohTs.append(ohT)
                    return ohTs

                cache = make_ohts(0)
                for g in range(NG):
                    ohTs = cache
                    if g + 1 < NG:
                        cache = make_ohts(g + 1)
                    og = outp.tile([128, GT, D], bf16, tag="og", name="og")
                    for j in range(GT):
                        t = g * GT + j
                        lo, hi = ranges[t]
                        pse = psE.tile([128, D], f32, tag="ps_e", name="ps_e")
                        for m in range(lo, hi + 1):
                            nc.tensor.matmul(
                                pse, lhsT=ohTs[j][:, m, :],
                                rhs=y_bf[:, m, :],
                                start=(m == lo), stop=(m == hi))
                        if t % 4 == 3:
                            nc.vector.tensor_copy(og[:, j, :], pse)
                        else:
                            nc.scalar.copy(out=og[:, j, :], in_=pse)
                        # store in half-group chunks: the final 0.5MB DMA is
                        # the kernel tail, so keep it small.
                        if j % (GT // 2) == GT // 2 - 1:
                            hf = j // (GT // 2)
                            base = g * GT + hf * (GT // 2)
                            nc.sync.dma_start(
                                out=out_d[base * 128:(base + GT // 2) * 128, :]
                                .rearrange("(n p) d -> p n d", p=128),
                                in_=og[:, hf * (GT // 2):(hf + 1) * (GT // 2), :])

    return nc


def _host_prep(inputs):
    """Shard + preprocess full inputs into 8 per-core input maps."""
    bf = ml_dtypes.bfloat16
    f8 = ml_dtypes.float8_e4m3fn
    tokens = np.asarray(inputs["tokens"], dtype=np.float32)
    seg = np.asarray(inputs["segment_ids"], dtype=np.int32)
    qkv_w = np.asarray(inputs["qkv_w"], dtype=np.float32)
    qkv_b = np.asarray(inputs["qkv_b"], dtype=np.float32)
    out_w = np.asarray(inputs["out_w"], dtype=np.float32)
    out_b = np.asarray(inputs["out_b"], dtype=np.float32)
    ln1_w = np.asarray(inputs["ln1_w"], dtype=np.float32)
    ln1_b = np.asarray(inputs["ln1_b"], dtype=np.float32)
    ln2_w = np.asarray(inputs["ln2_w"], dtype=np.float32)
    ln2_b = np.asarray(inputs["ln2_b"], dtype=np.float32)
    ff1_w = np.asarray(inputs["ff1_w"], dtype=np.float32)
    ff1_b = np.asarray(inputs["ff1_b"], dtype=np.float32)
    ff2_w = np.asarray(inputs["ff2_w"], dtype=np.float32)
    ff2_b = np.asarray(inputs["ff2_b"], dtype=np.float32)
    fln_w = np.asarray(inputs["fln_w"], dtype=np.float32)
    fln_b = np.asarray(inputs["fln_b"], dtype=np.float32)

    flags = (
        bool(np.any(qkv_b)),
        bool(np.any(out_b)),
        bool(np.any(ff1_b)),
        bool(np.any(ff2_b)),
        bool(np.any(ln1_w != 1.0) or np.any(ln1_b)),
        bool(np.any(ln2_w != 1.0) or np.any(ln2_b)),
        bool(np.any(fln_w != 1.0) or np.any(fln_b)),
    )

    # span-bound ranges: per token tile, union over batch of the contiguous
    # chunk-tile range its (sorted) segment ids cover.
    srt = np.all(np.diff(seg, axis=1) >= 0)
    if srt:
        lo = np.min(seg[:, ::128] // 128, axis=0)
        hi = np.max(seg[:, 127::128] // 128, axis=0)
    else:  # fallback: no structure assumed
        lo = np.zeros(NT, np.int64)
        hi = np.full(NT, CT - 1, np.int64)
    covered = set()
    for t in range(NT):
        covered.update(range(int(lo[t]), int(hi[t]) + 1))
    if covered != set(range(CT)):
        lo = np.zeros(NT, np.int64)
        hi = np.full(NT, CT - 1, np.int64)
    ranges = tuple((int(lo[t]), int(hi[t])) for t in range(NT))

    # shared (batch-independent) arrays
    shared = {
        "iota_row": np.broadcast_to(
            np.arange(C, dtype=np.float16)[None, :], (128, C)).copy(),
        "iota_col": (np.arange(CT, dtype=np.float32)[None, :] * 128
                     + np.arange(128, dtype=np.float32)[:, None]).astype(np.float32),
        "wqkvT": np.ascontiguousarray(qkv_w.transpose(0, 2, 1)).astype(bf),
        "woT": np.ascontiguousarray(out_w.transpose(0, 2, 1)).astype(bf),
        "w1T": np.ascontiguousarray(ff1_w.transpose(0, 2, 1)).astype(bf),
        "w2T": np.ascontiguousarray(ff2_w.transpose(0, 2, 1)).astype(bf),
    }
    (has_qkv_b, has_out_b, has_ff1_b, has_ff2_b,
     has_ln1, has_ln2, has_fln) = flags
    if has_qkv_b:
        shared["bqkv_c"] = np.ascontiguousarray(
            qkv_b[:, :1536].reshape(L, 12, 128).transpose(0, 2, 1))
        shared["vb_row"] = np.ascontiguousarray(qkv_b[:, 2 * D:3 * D][:, None, :])
    if has_ff1_b:
        shared["b1_c"] = np.ascontiguousarray(
            ff1_b.reshape(L, FT, 128).transpose(0, 2, 1))
    if has_out_b:
        shared["outb_row"] = np.ascontiguousarray(out_b[:, None, :])
    if has_ff2_b:
        shared["ff2b_row"] = np.ascontiguousarray(ff2_b[:, None, :])
    if has_ln1:
        shared["ln1w_row"] = np.ascontiguousarray(ln1_w[:, None, :])
        shared["ln1b_row"] = np.ascontiguousarray(ln1_b[:, None, :])
    if has_ln2:
        shared["ln2w_row"] = np.ascontiguousarray(ln2_w[:, None, :])
        shared["ln2b_row"] = np.ascontiguousarray(ln2_b[:, None, :])
    if has_fln:
        shared["flnw_row"] = np.ascontiguousarray(fln_w[None, :])
        shared["flnb_row"] = np.ascontiguousarray(fln_b[None, :])

    in_maps = []
    for b in range(B):
        m = dict(shared)
        m["tokens_bf"] = np.ascontiguousarray(tokens[b]).astype(bf)
        m["seg_col"] = np.ascontiguousarray(
            seg[b].reshape(NT, 128).T.astype(np.float32))
        m["seg_row"] = np.ascontiguousarray(seg[b].astype(np.float16)[None, :])
        counts = np.bincount(seg[b], minlength=C).astype(np.float32)
        m["inv_cnt"] = np.ascontiguousarray(
            (1.0 / np.maximum(counts, 1.0)).reshape(CT, 128).T)
        in_maps.append(m)
    return flags, ranges, in_maps


def kernel(**inputs) -> np.ndarray:
    from concourse.bass_utils import run_bass_kernel_spmd

    flags, ranges, in_maps = _host_prep(inputs)
    key = (flags, ranges)
    if key not in _CACHE:
        nc = _build(flags, ranges)
        if not nc.is_finalized():
            nc.finalize()
        _CACHE[key] = nc
    nc = _CACHE[key]
    res = run_bass_kernel_spmd(nc, in_maps, list(range(B)))
    return np.stack(
        [np.asarray(res.results[i]["out_bf"]).astype(np.float32)
         for i in range(B)], axis=0)
